# revision 43
# baseline (speedup 1.0000x reference)
"""DeepseekV3 MLA attention on 8 Trainium2 NeuronCores (Bass/Tile).

Sharding: core = (batch b, head-group g); b = core//4, g = core%4.
Each core handles 4 of 16 heads for one batch.
 - q_a / kv_a low-rank projections are sequence-sharded within each 4-core
   batch group (each core computes a 512-column chunk of q_a^T / kv_a^T),
   then AllGathered.  The kv gather fires right after the kv strip; the q
   gather is split into three row-chunk gathers, each fired as soon as its
   512-row strip is evicted, so gathers pipeline with stage-A compute.
 - Stage B runs the kv up-projection first (it only needs the early kv
   gather), hiding the q gathers behind it; all stage-B/D weights are
   prefetched into SBUF before the collectives start so the PE never
   stalls on DMA during a gather.
 - q_b / kv_b up-projections + causal attention computed per-core for its
   4 heads, everything in "transposed" (feature-on-partition) layout.
 - o-proj partial output per core ([S, H] fp16, contraction over the 512
   local v-dims); host sums the 4 partials per batch in fp32.

All matmul operands are fp16 (2-byte streaming runs the PE at full rate;
fp32 streams at half rate), accumulation is always fp32 in PSUM.  Softmax
uses exp(scale*s - 2) with no row-max subtraction (global shift cancels in
the ratio; score range is bounded well inside fp16), with the denominator
produced by a ones-vector matmul over the transposed probability tiles.
"""
import sys

sys.path.insert(0, "/opt/trn_rl_repo")

import numpy as np

import concourse.bass as bass
import concourse.tile as tile
from concourse import bacc, mybir
from concourse.bass_utils import run_bass_kernel_spmd

# ---- problem constants (hardcoded per contract) ----
B, S, H = 2, 2048, 2048
NH, D_NOPE, D_ROPE, D_V = 16, 128, 64, 128
D_QK = D_NOPE + D_ROPE
QLR, KVL = 1536, 512
SCALING = float(D_QK) ** -0.5
EXP_SHIFT = -2.0   # global exponent shift; cancels in softmax ratio
EPS = 1e-6
ROPE_THETA = 10000.0

NHL = 4            # heads per core
N_CORES = 8
SC = S // 4        # seq chunk per core for the a-projections (512)
P = 128
KV_ROWS = KVL + D_ROPE + 1   # 577: c_kv rows, k_rope rows, kv sumsq row
Q_ROWS = QLR + 1             # 1537: q_a rows, q sumsq row

f32 = mybir.dt.float32
f32r = mybir.dt.float32r
f16 = mybir.dt.float16
f8 = mybir.dt.float8e4
DR = mybir.MatmulPerfMode.DoubleRow
AF = mybir.ActivationFunctionType
RG = [[0, 1, 2, 3], [4, 5, 6, 7]]


def _build_program(use_collective=True):
    nc = bacc.Bacc("TRN2", target_bir_lowering=False, debug=False,
                   num_devices=N_CORES)

    def din(name, shape, dt=f16):
        return nc.dram_tensor(name, list(shape), dt, kind="ExternalInput").ap()

    # per-core inputs (host-prepped layouts, fp16)
    hT = din("hT", [H, S if not use_collective else SC])
    qawT_s = din("qawT_s", [16, 3, P, 512])      # q_a_w.T strips [k, mg, 128, 512]
    kvawT_s = din("kvawT_s", [16, P, 640])       # kv_a_w.T strips (cols padded 576->640)
    qbw_np = din("qbw_np", [QLR, NHL * D_NOPE])  # [1536, 512]
    qbw_rp = din("qbw_rp", [QLR, NHL * D_ROPE])  # [1536, 256]
    kvbw_k = din("kvbw_k", [KVL, NHL * D_NOPE])  # [512, 512]
    kvbw_v = din("kvbw_v", [KVL, NHL * D_V])     # [512, 512]
    owT = din("owT", [NHL * D_V, H])             # [512, 2048]
    cosP = din("cosP", [P, S], f32)              # vstack(cos.T, cos.T)
    sinP = din("sinP", [P, S], f32)
    r128t = din("r128t", [P, P])                 # rotate-half matrix (transposed, block-diag)
    triu = din("triu", [P, P])                   # causal mask for diagonal blocks
    ones_col = din("ones_col", [P, 1])           # lhsT for denominator matmul
    ones_row = din("ones_row", [1, P], f32)      # lhsT for partition-broadcast matmul (f32r)

    o_part = nc.dram_tensor("o_part", [S, H], f16, kind="ExternalOutput").ap()

    # scratch (fp16): kv block gathered first, q chunks pipelined after
    a_kv_loc = nc.dram_tensor("a_kv_loc", [KV_ROWS, SC], f16).ap()
    a_q_loc = nc.dram_tensor("a_q_loc", [Q_ROWS, SC], f16).ap()
    a_kv = nc.dram_tensor("a_kv", [4, KV_ROWS, SC], f16).ap()
    a_q0 = nc.dram_tensor("a_q0", [4, 512, SC], f16).ap()
    a_q1 = nc.dram_tensor("a_q1", [4, 1025, SC], f16).ap()

    with tile.TileContext(nc) as tc:
        _emit(tc, nc, use_collective, dict(
            hT=hT, qawT_s=qawT_s, kvawT_s=kvawT_s, qbw_np=qbw_np,
            qbw_rp=qbw_rp, kvbw_k=kvbw_k, kvbw_v=kvbw_v, owT=owT,
            cosP=cosP, sinP=sinP, r128t=r128t, triu=triu,
            ones_col=ones_col, ones_row=ones_row,
            o_part=o_part, a_kv_loc=a_kv_loc, a_q_loc=a_q_loc,
            a_kv=a_kv, a_q0=a_q0, a_q1=a_q1))
    nc.compile()
    return nc


def _emit(tc, nc, use_collective, t):
    from contextlib import ExitStack

    dma = nc.sync.dma_start
    MM = nc.tensor.matmul

    with ExitStack() as stack:
        ec = stack.enter_context

        consts = ec(tc.tile_pool(name="consts", bufs=1))
        ones_col_sb = consts.tile([P, 1], f16)
        dma(out=ones_col_sb, in_=t["ones_col"])
        ones_row_sb = consts.tile([1, P], f32r)
        dma(out=ones_row_sb, in_=t["ones_row"].bitcast(f32r))
        triu_sb = consts.tile([P, P], f16)
        dma(out=triu_sb, in_=t["triu"])
        shift_sb = consts.tile([P, 1], f32)
        nc.vector.memset(shift_sb[:], EXP_SHIFT)
        eps_sb = consts.tile([1, 1], f32)
        nc.vector.memset(eps_sb[:], EPS)
        ones_11 = consts.tile([1, 1], f16)
        nc.vector.memset(ones_11[:], 1.0)

        # stage-B/D weight tiles (DMAs fired from inside stage A)
        pre_w = ec(tc.tile_pool(name="pre_w", bufs=1))
        kvbw_k_sb = pre_w.tile([P, 4, NHL * D_NOPE], f16)
        kvbw_v_sb = pre_w.tile([P, 4, NHL * D_V], f16)
        qbw_np_sb = pre_w.tile([P, 12, NHL * D_NOPE], f16)
        qbw_rp_sb = pre_w.tile([P, 12, NHL * D_ROPE], f16)
        ow_sb = pre_w.tile([P, 4, H], f16)

        # ================ Stage A: a-projections (kv strip first) ================
        n_chunk = 1 if use_collective else 4
        with tc.tile_pool(name="a_sb", bufs=1) as a_sb_pool, \
             tc.tile_pool(name="a_wkv", bufs=16) as a_wkv_pool, \
             tc.tile_pool(name="a_wq", bufs=24) as a_wq_pool, \
             tc.tile_pool(name="a_ev", bufs=10) as a_ev_pool, \
             tc.tile_pool(name="a_ps", bufs=1, space="PSUM") as a_ps_pool, \
             tc.tile_pool(name="a_ssq", bufs=1, space="PSUM") as a_ssq_pool:
            for cs in range(n_chunk):
                if use_collective:
                    hsrc = t["hT"]
                else:
                    hsrc = t["hT"][:, cs * SC:(cs + 1) * SC]
                # separate tiles per strip so the first matmul starts as soon
                # as strip 0 lands (one tile would add a whole-2MB dependency)
                hT_sb = []
                for k in range(16):
                    hk = a_sb_pool.tile([P, SC], f16, tag=f"hT{k}")
                    dma(out=hk, in_=hsrc[k * P:(k + 1) * P, :])
                    hT_sb.append(hk)
                kv_dst = t["a_kv_loc"] if use_collective else t["a_kv"][cs]
                ssq_ps = [a_ssq_pool.tile([1, SC], f32, tag=f"ssq{i}", name=f"ssq{i}")
                          for i in range(2)]

                def evict(psum, dst, dst_row, ssq_grp, ssq_start, ssq_stop, rows=P):
                    val = a_ev_pool.tile([P, SC], f16, tag="val")
                    nc.vector.tensor_copy(val[0:rows, :], psum[0:rows, :])
                    dma(out=dst[dst_row:dst_row + rows, :], in_=val[0:rows, :])
                    if ssq_grp is not None:
                        sq = a_ev_pool.tile([P, SC], f16, tag="sq")
                        nc.scalar.activation(sq[:], psum[:], AF.Square)
                        MM(ssq_ps[ssq_grp][:], lhsT=ones_col_sb[:], rhs=sq[:],
                           start=ssq_start, stop=ssq_stop)

                # ---- kv strip (5 m-tiles: 4 c_kv + 1 rope, 640 cols) ----
                psums = [a_ps_pool.tile([P, SC], f32, tag=f"aps{i}", name=f"aps{i}")
                         for i in range(5)]
                for k in range(16):
                    wt = a_wkv_pool.tile([P, 640], f16, tag="awkv")
                    dma(out=wt, in_=t["kvawT_s"][k])
                    for ml in range(5):
                        MM(psums[ml][:], lhsT=wt[:, ml * P:(ml + 1) * P],
                           rhs=hT_sb[k][:], start=(k == 0), stop=(k == 15))
                for ml in range(4):
                    evict(psums[ml], kv_dst, ml * P, 1, ml == 0, ml == 3)
                evict(psums[4], kv_dst, KVL, None, False, False, rows=D_ROPE)
                # local 1/rms scale for the kv chunk (gathered as fp16 row)
                rs = a_ev_pool.tile([1, SC], f32, tag="rs")
                nc.scalar.activation(rs[:], ssq_ps[1][:], AF.Sqrt,
                                     scale=1.0 / KVL, bias=eps_sb[:])
                rr = a_ev_pool.tile([1, SC], f32, tag="rr")
                nc.vector.reciprocal(rr[:], rs[:])
                s_sb = a_ev_pool.tile([1, SC], f16, tag="ssqe")
                nc.vector.tensor_copy(s_sb[:], rr[:])
                dma(out=kv_dst[KVL + D_ROPE:KVL + D_ROPE + 1, :], in_=s_sb[:])
                if use_collective and cs == 0:
                    nc.gpsimd.collective_compute(
                        "AllGather", mybir.AluOpType.bypass, replica_groups=RG,
                        ins=[t["a_kv_loc"].opt()], outs=[t["a_kv"].opt()])
                    # up-proj weight preloads land in the stage-A DMA slack,
                    # before the gathers own the fabric
                    dma(out=kvbw_k_sb, in_=t["kvbw_k"].rearrange("(t p) n -> p t n", p=P))
                    dma(out=kvbw_v_sb, in_=t["kvbw_v"].rearrange("(t p) n -> p t n", p=P))
                    dma(out=qbw_np_sb, in_=t["qbw_np"].rearrange("(t p) n -> p t n", p=P))
                    dma(out=qbw_rp_sb, in_=t["qbw_rp"].rearrange("(t p) n -> p t n", p=P))

                # ---- q strips (3 x 4 m-tiles); chunk gathers: mg0, then mg1+mg2 ----
                for mg in range(3):
                    if use_collective:
                        q_dst, qrow0 = t["a_q_loc"], mg * 512
                    elif mg == 0:
                        q_dst, qrow0 = t["a_q0"][cs], 0
                    else:
                        q_dst, qrow0 = t["a_q1"][cs], (mg - 1) * 512
                    psums = [a_ps_pool.tile([P, SC], f32, tag=f"aps{i}", name=f"aps{i}")
                             for i in range(4)]
                    for k in range(16):
                        wt = a_wq_pool.tile([P, 512], f16, tag="awq")
                        dma(out=wt, in_=t["qawT_s"][k, mg])
                        for ml in range(4):
                            MM(psums[ml][:], lhsT=wt[:, ml * P:(ml + 1) * P],
                               rhs=hT_sb[k][:], start=(k == 0), stop=(k == 15))
                    for ml in range(4):
                        m = mg * 4 + ml
                        evict(psums[ml], q_dst, qrow0 + ml * P, 0, m == 0, m == 11)
                    if use_collective and cs == 0 and mg == 0:
                        nc.gpsimd.collective_compute(
                            "AllGather", mybir.AluOpType.bypass, replica_groups=RG,
                            ins=[t["a_q_loc"][0:512, :].opt()],
                            outs=[t["a_q0"].opt()])
                rs = a_ev_pool.tile([1, SC], f32, tag="rs")
                nc.scalar.activation(rs[:], ssq_ps[0][:], AF.Sqrt,
                                     scale=1.0 / QLR, bias=eps_sb[:])
                rr = a_ev_pool.tile([1, SC], f32, tag="rr")
                nc.vector.reciprocal(rr[:], rs[:])
                s_sb = a_ev_pool.tile([1, SC], f16, tag="ssqe")
                nc.vector.tensor_copy(s_sb[:], rr[:])
                if use_collective:
                    dma(out=t["a_q_loc"][QLR:QLR + 1, :], in_=s_sb[:])
                    if cs == 0:
                        nc.gpsimd.collective_compute(
                            "AllGather", mybir.AluOpType.bypass, replica_groups=RG,
                            ins=[t["a_q_loc"][512:1537, :].opt()],
                            outs=[t["a_q1"].opt()])
                else:
                    dma(out=t["a_q1"][cs][1024:1025, :], in_=s_sb[:])

        # ---------------- persistent pools (post stage A) ----------------
        qk_stack = ExitStack()   # closed after stage C
        b_stack = ExitStack()    # closed after stage B
        pers_qk = qk_stack.enter_context(tc.tile_pool(name="pers_qk", bufs=1))
        pers_v = qk_stack.enter_context(tc.tile_pool(name="pers_v", bufs=1))
        q_npT = [pers_qk.tile([P, S], f16, tag=f"qnp{m}", name=f"qnp{m}") for m in range(NHL)]
        k_npT = [pers_qk.tile([P, S], f16, tag=f"knp{h}", name=f"knp{h}") for h in range(NHL)]
        # rope side in fp8 for DoubleRow score matmuls: k packs hold
        # (value, residual) plane pairs per 128-col block; _lo has the rope
        # dims on partitions 0-63 (even heads), _hi on 64-127 (odd heads),
        # zeros elsewhere so the off-half of the q operand is masked out.
        krp8_lo = pers_qk.tile([P, 16, 2, P], f8, tag="krp8l", name="krp8l")
        krp8_hi = pers_qk.tile([P, 16, 2, P], f8, tag="krp8h", name="krp8h")
        nc.vector.memset(krp8_lo[64:P, :, :, :], 0.0)
        nc.vector.memset(krp8_hi[0:64, :, :, :], 0.0)
        qrp8 = [pers_qk.tile([P, S], f8, tag=f"qrp8{m}", name=f"qrp8{m}")
                for m in range(2)]
        v_t = [pers_v.tile([P, NHL * D_V], f16, tag=f"v{st}", name=f"v{st}") for st in range(16)]

        bconsts = b_stack.enter_context(tc.tile_pool(name="bconsts", bufs=1))
        cosP_sb = bconsts.tile([P, S], f32)
        dma(out=cosP_sb, in_=t["cosP"])
        sinP_sb = bconsts.tile([P, S], f32)
        dma(out=sinP_sb, in_=t["sinP"])
        r128_sb = bconsts.tile([P, P], f16)
        dma(out=r128_sb, in_=t["r128t"])

        # ======== kv rms scales: broadcast-load the gathered fp16 scale row ====
        rpool = b_stack.enter_context(tc.tile_pool(name="rpool", bufs=1))
        kv_dt = t["a_kv"].tensor
        kv_roff = (KVL + D_ROPE) * SC
        bcast_rk = rpool.tile([P, S], f16, tag="brk")
        dma(out=bcast_rk.rearrange("p (c s) -> p c s", c=4),
            in_=bass.AP(tensor=kv_dt, offset=kv_roff,
                        ap=[[0, P], [KV_ROWS * SC, 4], [1, SC]]))
        # rkvT[p, st] = scale(seq st*128+p): row-to-column via tiny
        # K=1 transpose matmuls (an element-strided DMA here costs ~23us)
        rkv_row = rpool.tile([1, S], f16, tag="rkvrow")
        for c in range(4):
            dma(out=rkv_row[0:1, c * SC:(c + 1) * SC],
                in_=t["a_kv"][c, KVL + D_ROPE:KVL + D_ROPE + 1, :])
        rkvT = rpool.tile([P, 16], f32, tag="rkvT")
        with tc.tile_pool(name="rkv_ps", bufs=1, space="PSUM") as rkv_psp:
            rkv_ps = rkv_psp.tile([P, 16], f32)
            for st in range(16):
                MM(rkv_ps[:, st:st + 1], lhsT=rkv_row[0:1, st * P:(st + 1) * P],
                   rhs=ones_11[:], start=True, stop=True)
            nc.vector.tensor_copy(rkvT[:], rkv_ps[:])

        # ================ Stage B kv-pass (overlaps the q gathers) ============
        with tc.tile_pool(name="bk_s", bufs=4) as bk_s, \
             tc.tile_pool(name="bk_t", bufs=2) as bk_t, \
             tc.tile_pool(name="bk_ps", bufs=1, space="PSUM") as bk_ps, \
             tc.tile_pool(name="bk_ps2", bufs=2, space="PSUM") as bk_ps2:
            # hoist all chunk loads: they only need the kv gather, so they
            # land before the q gathers hog the fabric
            kva_sbs = []
            for c in range(4):
                kva_sb = bk_s.tile([P, 4, SC], f16, tag="kva")
                dma(out=kva_sb,
                    in_=t["a_kv"][c, 0:KVL, :].rearrange("(t p) s -> p t s", p=P))
                kva_sbs.append(kva_sb)
            for c in range(4):
                cols = slice(c * SC, (c + 1) * SC)
                kva_sb = kva_sbs[c]
                kps = [bk_ps.tile([P, SC], f32, tag=f"kps{h}", name=f"kps{h}")
                       for h in range(NHL)]
                for f in range(4):
                    for h in range(NHL):
                        MM(kps[h][:], lhsT=kvbw_k_sb[:, f, h * P:(h + 1) * P],
                           rhs=kva_sb[:, f, :], start=(f == 0), stop=(f == 3))
                for h in range(NHL):
                    nc.vector.tensor_mul(k_npT[h][:, cols], kps[h][:], bcast_rk[:, cols])
                for stl in range(4):
                    st = 4 * c + stl
                    vps = bk_ps2.tile([P, NHL * D_V], f32, tag="vps")
                    for f in range(4):
                        MM(vps[:], lhsT=kva_sb[:, f, stl * P:(stl + 1) * P],
                           rhs=kvbw_v_sb[:, f, :], start=(f == 0), stop=(f == 3))
                    nc.vector.tensor_scalar_mul(v_t[st][:], vps[:], rkvT[:, st:st + 1])
                # k_rope (loaded duplicated into both 64-partition halves)
                rp_f = bk_t.tile([P, SC], f16, tag="rp")
                kv_t_ = t["a_kv"].tensor
                rope_off = (c * KV_ROWS + KVL) * SC
                dma(out=rp_f[:],
                    in_=bass.AP(tensor=kv_t_, offset=rope_off,
                                ap=[[0, 2], [SC, D_ROPE], [1, SC]]))
                rot_ps = bk_ps2.tile([P, SC], f32, tag="rotk")
                MM(rot_ps[:], lhsT=r128_sb[:], rhs=rp_f[:], start=True, stop=True)
                t1 = bk_t.tile([P, SC], f32, tag="t1k")
                nc.vector.tensor_mul(t1[:], rot_ps[:], sinP_sb[:, cols])
                t2 = bk_t.tile([P, SC], f32, tag="t2k")
                nc.vector.tensor_mul(t2[:], rp_f[:], cosP_sb[:, cols])
                rope32 = bk_t.tile([P, SC], f32, tag="rope32")
                nc.vector.tensor_add(rope32[:], t1[:], t2[:])
                # fp8 packs: plane0 = value, plane1 = residual (k-side
                # compensation halves the rope quantization error)
                deq = bk_t.tile([P, SC], f32, tag="deq")
                res = bk_t.tile([P, SC], f32, tag="res")
                for pack, rlo in ((krp8_lo, slice(0, 64)), (krp8_hi, slice(64, P))):
                    dst0 = pack[rlo, 4 * c:4 * c + 4, 0, :]
                    src = rope32[rlo, :].rearrange("p (j m) -> p j m", j=4)
                    nc.vector.tensor_copy(dst0, src)
                    nc.vector.tensor_copy(deq[rlo, :].rearrange("p (j m) -> p j m", j=4), dst0)
                    nc.vector.tensor_sub(res[rlo, :], rope32[rlo, :], deq[rlo, :])
                    nc.vector.tensor_copy(pack[rlo, 4 * c:4 * c + 4, 1, :],
                                          res[rlo, :].rearrange("p (j m) -> p j m", j=4))

        # ======== q rms scales: broadcast-load the gathered fp16 scale row =====
        bcast_rq = rpool.tile([P, S], f16, tag="brq")
        dma(out=bcast_rq.rearrange("p (c s) -> p c s", c=4),
            in_=bass.AP(tensor=t["a_q1"].tensor, offset=1024 * SC,
                        ap=[[0, P], [1025 * SC, 4], [1, SC]]))

        # ================ Stage B q-pass ================
        with tc.tile_pool(name="bq_s", bufs=3) as bq_s, \
             tc.tile_pool(name="bq_t", bufs=2) as bq_t, \
             tc.tile_pool(name="bq_ps", bufs=1, space="PSUM") as bq_ps, \
             tc.tile_pool(name="bq_ps2", bufs=2, space="PSUM") as bq_ps2:
            for c in range(4):
                cols = slice(c * SC, (c + 1) * SC)
                qps = [bq_ps.tile([P, SC], f32, tag=f"qps{m}", name=f"qps{m}")
                       for m in range(6)]
                for f in range(12):
                    qa_f = bq_s.tile([P, SC], f16, tag="qa")
                    if f < 4:
                        qsrc = t["a_q0"][c, f * P:(f + 1) * P, :]
                    else:
                        qsrc = t["a_q1"][c, (f - 4) * P:(f - 3) * P, :]
                    dma(out=qa_f, in_=qsrc)
                    for m in range(4):
                        MM(qps[m][:], lhsT=qbw_np_sb[:, f, m * P:(m + 1) * P],
                           rhs=qa_f[:], start=(f == 0), stop=(f == 11))
                    for m2 in range(2):
                        MM(qps[4 + m2][:], lhsT=qbw_rp_sb[:, f, m2 * P:(m2 + 1) * P],
                           rhs=qa_f[:], start=(f == 0), stop=(f == 11))
                for m in range(4):
                    nc.vector.tensor_mul(q_npT[m][:, cols], qps[m][:], bcast_rq[:, cols])
                for m2 in range(2):
                    x_sb = bq_t.tile([P, SC], f16, tag="x")
                    nc.vector.tensor_mul(x_sb[:], qps[4 + m2][:], bcast_rq[:, cols])
                    rot_ps = bq_ps2.tile([P, SC], f32, tag="rot")
                    MM(rot_ps[:], lhsT=r128_sb[:], rhs=x_sb[:], start=True, stop=True)
                    t1 = bq_t.tile([P, SC], f32, tag="t1")
                    nc.vector.tensor_mul(t1[:], rot_ps[:], sinP_sb[:, cols])
                    t2 = bq_t.tile([P, SC], f32, tag="t2")
                    nc.vector.tensor_mul(t2[:], x_sb[:], cosP_sb[:, cols])
                    nc.vector.tensor_add(qrp8[m2][:, cols], t1[:], t2[:])

        b_stack.close()

        # o-proj weights stream in during stage C, well before stage D
        dma(out=ow_sb, in_=t["owT"].rearrange("(t p) n -> p t n", p=P))

        # ================ Stage C: attention ================
        pers_at = ec(tc.tile_pool(name="pers_at", bufs=1, side="right"))
        at_onT = [pers_at.tile([P, S], f16, tag=f"aon{h}", name=f"aon{h}") for h in range(NHL)]
        with tc.tile_pool(name="c_pt", bufs=4) as c_pt, \
             tc.tile_pool(name="c_r", bufs=3) as c_r, \
             tc.tile_pool(name="c_sc", bufs=2, space="PSUM") as c_sc, \
             tc.tile_pool(name="c_at", bufs=3, space="PSUM") as c_at, \
             tc.tile_pool(name="c_dn", bufs=1, space="PSUM") as c_dn, \
             tc.tile_pool(name="c_bc", bufs=2, space="PSUM") as c_bc:
            # all in-flight denominators share one PSUM bank: rotating
            # [1,512] slots at base partitions 0/32/64
            dn_all = c_dn.tile([65, 512], f32, tag="dn", name="dn_all")

            def emit_norm(h, qcols, at_ps, dn_ps):
                rec = c_r.tile([1, 512], f32r, tag="rec")
                with nc.allow_low_precision(reason="f32r carries full fp32 bits"):
                    nc.vector.reciprocal(rec[:], dn_ps)
                bc_ps = c_bc.tile([P, 512], f32, tag="bc")
                MM(bc_ps[:], lhsT=ones_row_sb[:], rhs=rec[:], start=True, stop=True)
                bc_sb = c_r.tile([P, 512], f32, tag="bcs")
                nc.scalar.copy(bc_sb[:], bc_ps[:])
                nc.vector.tensor_mul(at_onT[h][:, qcols], at_ps[:], bc_sb[:])

            pending = None
            for h in range(NHL):
                krp8 = krp8_lo if h % 2 == 0 else krp8_hi
                qrp8_t = qrp8[h // 2]
                for Q in range(4):
                    qcols = slice(Q * 512, (Q + 1) * 512)
                    at_ps = c_at.tile([P, 512], f32, tag="at")
                    slot = ((h * 4 + Q) % 3) * 32
                    dn_ps = dn_all[slot:slot + 1, :]
                    jmax = 4 * Q + 3
                    for j in range(jmax + 1):
                        jp = j - 4 * Q
                        lo = max(jp, 0) * P
                        qsl = slice(Q * 512 + lo, (Q + 1) * 512)
                        ksl = slice(j * P, (j + 1) * P)
                        sc_ps = c_sc.tile([P, 512], f32, tag="sc")
                        MM(sc_ps[:, lo:], lhsT=k_npT[h][:, ksl], rhs=q_npT[h][:, qsl],
                           start=True, stop=False)
                        # rope part: fp8 DoubleRow, (value, residual) planes on
                        # the k side, q plane streamed twice via 0-stride AP
                        q_ap = qrp8_t[:, qsl]
                        q_dr = bass.AP(tensor=q_ap.tensor, offset=q_ap.offset,
                                       ap=[q_ap.ap[0], [0, 2], [1, 512 - lo]])
                        MM(sc_ps[:, lo:], lhsT=krp8[:, j, :, :], rhs=q_dr,
                           start=False, stop=True, perf_mode=DR,
                           skip_group_check=True)
                        pt = c_pt.tile([P, 512], f16, tag="pt")
                        nc.scalar.activation(pt[:, lo:], sc_ps[:, lo:], AF.Exp,
                                             scale=SCALING, bias=shift_sb[:])
                        if jp >= 0:
                            nc.vector.tensor_mul(pt[:, lo:lo + P], pt[:, lo:lo + P],
                                                 triu_sb[:])
                        MM(at_ps[:, lo:], lhsT=v_t[j][:, h * D_V:(h + 1) * D_V],
                           rhs=pt[:, lo:], start=(j == 0), stop=(j == jmax))
                        MM(dn_ps[:, lo:], lhsT=ones_col_sb[:], rhs=pt[:, lo:],
                           start=(j == 0), stop=(j == jmax))
                    # normalization for the PREVIOUS (h,Q): its bc matmul sits
                    # behind this (h,Q)'s scores in the in-order PE queue, so
                    # the 3.4us vector reciprocal never stalls the PE
                    if pending is not None:
                        emit_norm(*pending)
                    pending = (h, qcols, at_ps, dn_ps)
            emit_norm(*pending)

        qk_stack.close()

        # ================ Stage D: o-proj partial ================
        with tc.tile_pool(name="d_o", bufs=2) as d_o, \
             tc.tile_pool(name="d_ps", bufs=2, space="PSUM") as d_ps:
            for qt in range(16):
                out_sb = d_o.tile([P, H], f16, tag="out")
                for hc in range(4):
                    psum = d_ps.tile([P, 512], f32, tag="dps")
                    for f in range(4):
                        MM(psum[:], lhsT=at_onT[f][:, qt * P:(qt + 1) * P],
                           rhs=ow_sb[:, f, hc * 512:(hc + 1) * 512],
                           start=(f == 0), stop=(f == 3))
                    nc.scalar.copy(out_sb[:, hc * 512:(hc + 1) * 512], psum[:])
                dma(out=t["o_part"][qt * P:(qt + 1) * P, :], in_=out_sb[:])


# ---------------- host side ----------------
_CACHED = {}


def _get_program(use_collective=True):
    key = use_collective
    if key not in _CACHED:
        _CACHED[key] = _build_program(use_collective)
    return _CACHED[key]


def _host_consts():
    inv_freq = 1.0 / (ROPE_THETA ** (np.arange(0, D_ROPE, 2, dtype=np.float32) / D_ROPE))
    ti = np.arange(S, dtype=np.float32)
    ang = np.outer(ti, inv_freq)
    emb = np.concatenate([ang, ang], axis=-1)          # [S, 64]
    cosT = np.cos(emb).T.astype(np.float32)            # [64, S]
    sinT = np.sin(emb).T.astype(np.float32)
    cosP = np.vstack([cosT, cosT])                     # [128, S]
    sinP = np.vstack([sinT, sinT])
    r64 = np.zeros((D_ROPE, D_ROPE), np.float16)
    hlf = D_ROPE // 2
    for i in range(hlf):
        r64[i, i + hlf] = -1.0
        r64[i + hlf, i] = 1.0
    r128 = np.zeros((P, P), np.float16)
    r128[:D_ROPE, :D_ROPE] = r64
    r128[D_ROPE:, D_ROPE:] = r64
    r128t = np.ascontiguousarray(r128.T)
    kk, qq = np.meshgrid(np.arange(P), np.arange(P), indexing="ij")
    triu = (kk <= qq).astype(np.float16)
    return cosP, sinP, r128t, triu


def make_in_maps(hidden_states, q_a_w, q_a_ln_w, q_b_w, kv_a_w, kv_a_ln_w,
                 kv_b_w, o_w, use_collective=True):
    f, f16_ = np.float32, np.float16
    hidden_states = np.asarray(hidden_states, f)
    q_b_eff = (np.asarray(q_b_w, f) * np.asarray(q_a_ln_w, f)[None, :]).astype(f16_)
    kv_b_eff = (np.asarray(kv_b_w, f) * np.asarray(kv_a_ln_w, f)[None, :]).astype(f16_)
    qawT = np.asarray(q_a_w, f).T.astype(f16_)         # [H, QLR]
    kvawT_pad = np.zeros((H, 5 * P), f16_)
    kvawT_pad[:, :KVL + D_ROPE] = np.asarray(kv_a_w, f).T.astype(f16_)
    qawT_s = np.ascontiguousarray(
        qawT.reshape(16, P, 3, 512).transpose(0, 2, 1, 3))
    kvawT_s = np.ascontiguousarray(kvawT_pad.reshape(16, P, 640))
    cosP, sinP, r128t, triu = _host_consts()
    ones_col = np.ones((P, 1), f16_)
    ones_row = np.ones((1, P), f)

    in_maps = []
    for core in range(N_CORES):
        b, g = divmod(core, 4)
        heads = range(NHL * g, NHL * (g + 1))
        if use_collective:
            hT = np.ascontiguousarray(hidden_states[b, g * SC:(g + 1) * SC, :].T.astype(f16_))
        else:
            hT = np.ascontiguousarray(hidden_states[b].T.astype(f16_))
        qbw_np = np.ascontiguousarray(np.concatenate(
            [q_b_eff[D_QK * hh:D_QK * hh + D_NOPE] for hh in heads], 0).T)
        qbw_rp = np.ascontiguousarray(np.concatenate(
            [q_b_eff[D_QK * hh + D_NOPE:D_QK * (hh + 1)] for hh in heads], 0).T)
        kvbw_k = np.ascontiguousarray(np.concatenate(
            [kv_b_eff[(D_NOPE + D_V) * hh:(D_NOPE + D_V) * hh + D_NOPE]
             for hh in heads], 0).T)
        kvbw_v = np.ascontiguousarray(np.concatenate(
            [kv_b_eff[(D_NOPE + D_V) * hh + D_NOPE:(D_NOPE + D_V) * (hh + 1)]
             for hh in heads], 0).T)
        owT = np.ascontiguousarray(
            np.asarray(o_w, f)[:, g * NHL * D_V:(g + 1) * NHL * D_V].T.astype(f16_))
        in_maps.append(dict(
            hT=hT, qawT_s=qawT_s, kvawT_s=kvawT_s, qbw_np=qbw_np,
            qbw_rp=qbw_rp, kvbw_k=kvbw_k, kvbw_v=kvbw_v, owT=owT,
            cosP=cosP, sinP=sinP, r128t=r128t, triu=triu,
            ones_col=ones_col, ones_row=ones_row))
    return in_maps


def kernel(**inputs):
    use_collective = True
    nc = _get_program(use_collective)
    in_maps = make_in_maps(**inputs, use_collective=use_collective)
    res = run_bass_kernel_spmd(nc, in_maps, core_ids=list(range(N_CORES)))
    out = np.zeros((B, S, H), np.float32)
    for core in range(N_CORES):
        b = core // 4
        out[b] += res.results[core]["o_part"].astype(np.float32)
    return out


# revision 50
# speedup vs baseline: 1.1482x; 1.1482x over previous
"""DeepseekV3 MLA attention on 8 Trainium2 NeuronCores (Bass/Tile).

Sharding: core = (batch b, head-group g); b = core//4, g = core%4.
Each core handles 4 of 16 heads for one batch.
 - q_a / kv_a low-rank projections are sequence-sharded within each 4-core
   batch group (each core computes a 512-column chunk of q_a^T / kv_a^T),
   then AllGathered.  The kv gather fires right after the kv strip; the q
   gather is split into three row-chunk gathers, each fired as soon as its
   512-row strip is evicted, so gathers pipeline with stage-A compute.
 - Stage B runs the kv up-projection first (it only needs the early kv
   gather), hiding the q gathers behind it; all stage-B/D weights are
   prefetched into SBUF before the collectives start so the PE never
   stalls on DMA during a gather.
 - q_b / kv_b up-projections + causal attention computed per-core for its
   4 heads, everything in "transposed" (feature-on-partition) layout.
 - o-proj partial output per core ([S, H] fp16, contraction over the 512
   local v-dims); host sums the 4 partials per batch in fp32.

All matmul operands are fp16 (2-byte streaming runs the PE at full rate;
fp32 streams at half rate), accumulation is always fp32 in PSUM.  Softmax
uses exp(scale*s - 2) with no row-max subtraction (global shift cancels in
the ratio; score range is bounded well inside fp16), with the denominator
produced by a ones-vector matmul over the transposed probability tiles.
"""
import sys

sys.path.insert(0, "/opt/trn_rl_repo")

import numpy as np

import concourse.bass as bass
import concourse.tile as tile
from concourse import bacc, mybir
from concourse.bass_utils import run_bass_kernel_spmd

# ---- problem constants (hardcoded per contract) ----
B, S, H = 2, 2048, 2048
NH, D_NOPE, D_ROPE, D_V = 16, 128, 64, 128
D_QK = D_NOPE + D_ROPE
QLR, KVL = 1536, 512
SCALING = float(D_QK) ** -0.5
EXP_SHIFT = -2.0   # global exponent shift; cancels in softmax ratio
EPS = 1e-6
ROPE_THETA = 10000.0

NHL = 4            # heads per core
N_CORES = 8
SC = S // 4        # seq chunk per core for the a-projections (512)
P = 128
KV_ROWS = KVL + D_ROPE + 1   # 577: c_kv rows, k_rope rows, kv sumsq row
Q_ROWS = QLR + 1             # 1537: q_a rows, q sumsq row

f32 = mybir.dt.float32
f32r = mybir.dt.float32r
f16 = mybir.dt.float16
f8 = mybir.dt.float8e4
DR = mybir.MatmulPerfMode.DoubleRow
AF = mybir.ActivationFunctionType
RG = [[0, 1, 2, 3], [4, 5, 6, 7]]


def _build_program(use_collective=True):
    nc = bacc.Bacc("TRN2", target_bir_lowering=False, debug=False,
                   num_devices=N_CORES)

    def din(name, shape, dt=f16):
        return nc.dram_tensor(name, list(shape), dt, kind="ExternalInput").ap()

    # per-core inputs (host-prepped layouts, fp16)
    hT = din("hT", [H, S if not use_collective else SC])
    qawT_s = din("qawT_s", [16, 3, P, 512])      # q_a_w.T strips [k, mg, 128, 512]
    kvawT_s = din("kvawT_s", [16, P, 640])       # kv_a_w.T strips (cols padded 576->640)
    qbw_np = din("qbw_np", [QLR, NHL * D_NOPE])  # [1536, 512]
    qbw_rp = din("qbw_rp", [QLR, NHL * D_ROPE])  # [1536, 256]
    kvbw_k = din("kvbw_k", [KVL, NHL * D_NOPE])  # [512, 512]
    kvbw_v = din("kvbw_v", [KVL, NHL * D_V])     # [512, 512]
    owT = din("owT", [NHL * D_V, H])             # [512, 2048]
    cosP = din("cosP", [P, S], f32)              # vstack(cos.T, cos.T)
    sinP = din("sinP", [P, S], f32)
    r128t = din("r128t", [P, P])                 # rotate-half matrix (transposed, block-diag)
    triu = din("triu", [P, P])                   # causal mask for diagonal blocks
    ones_col = din("ones_col", [P, 1])           # lhsT for denominator matmul
    ones_row = din("ones_row", [1, P], f32)      # lhsT for partition-broadcast matmul (f32r)

    o_part = nc.dram_tensor("o_part", [S, H], f16, kind="ExternalOutput").ap()

    # scratch (fp16): kv block gathered first, q chunks pipelined after
    a_kv_loc = nc.dram_tensor("a_kv_loc", [KV_ROWS, SC], f16).ap()
    a_q_loc = nc.dram_tensor("a_q_loc", [1538, SC], f8).ap()
    a_kv = nc.dram_tensor("a_kv", [4, KV_ROWS, SC], f16).ap()
    a_q0 = nc.dram_tensor("a_q0", [4, 512, SC], f8).ap()
    a_q1 = nc.dram_tensor("a_q1", [4, 1026, SC], f8).ap()

    with tile.TileContext(nc) as tc:
        _emit(tc, nc, use_collective, dict(
            hT=hT, qawT_s=qawT_s, kvawT_s=kvawT_s, qbw_np=qbw_np,
            qbw_rp=qbw_rp, kvbw_k=kvbw_k, kvbw_v=kvbw_v, owT=owT,
            cosP=cosP, sinP=sinP, r128t=r128t, triu=triu,
            ones_col=ones_col, ones_row=ones_row,
            o_part=o_part, a_kv_loc=a_kv_loc, a_q_loc=a_q_loc,
            a_kv=a_kv, a_q0=a_q0, a_q1=a_q1))
    nc.compile()
    return nc


def _emit(tc, nc, use_collective, t):
    from contextlib import ExitStack

    dma = nc.sync.dma_start
    MM = nc.tensor.matmul

    with ExitStack() as stack:
        ec = stack.enter_context

        consts = ec(tc.tile_pool(name="consts", bufs=1))
        ones_col_sb = consts.tile([P, 1], f16)
        dma(out=ones_col_sb, in_=t["ones_col"])
        ones_row_sb = consts.tile([1, P], f32r)
        dma(out=ones_row_sb, in_=t["ones_row"].bitcast(f32r))
        triu_sb = consts.tile([P, P], f16)
        dma(out=triu_sb, in_=t["triu"])
        shift_sb = consts.tile([P, 1], f32)
        nc.vector.memset(shift_sb[:], EXP_SHIFT)
        eps_sb = consts.tile([1, 1], f32)
        nc.vector.memset(eps_sb[:], EPS)
        ones_11 = consts.tile([1, 1], f16)
        nc.vector.memset(ones_11[:], 1.0)
        ones_row16 = consts.tile([1, P], f16)
        nc.vector.memset(ones_row16[:], 1.0)

        # stage-B/D weight tiles (DMAs fired from inside stage A)
        pre_w = ec(tc.tile_pool(name="pre_w", bufs=1))
        kvbw_k_sb = pre_w.tile([P, 4, NHL * D_NOPE], f16)
        kvbw_v_sb = pre_w.tile([P, 4, NHL * D_V], f16)
        qbw_np_sb = pre_w.tile([P, 12, NHL * D_NOPE], f16)
        qbw_rp_sb = pre_w.tile([P, 12, NHL * D_ROPE], f16)
        ow_sb = pre_w.tile([P, 4, H], f16)

        # ================ Stage A: a-projections (kv strip first) ================
        n_chunk = 1 if use_collective else 4
        with tc.tile_pool(name="a_sb", bufs=1) as a_sb_pool, \
             tc.tile_pool(name="a_wkv", bufs=16) as a_wkv_pool, \
             tc.tile_pool(name="a_wq", bufs=24) as a_wq_pool, \
             tc.tile_pool(name="a_ev", bufs=6) as a_ev_pool, \
             tc.tile_pool(name="a_row", bufs=2) as a_row_pool, \
             tc.tile_pool(name="a_ps", bufs=1, space="PSUM") as a_ps_pool, \
             tc.tile_pool(name="a_ssq", bufs=1, space="PSUM") as a_ssq_pool:
            for cs in range(n_chunk):
                if use_collective:
                    hsrc = t["hT"]
                else:
                    hsrc = t["hT"][:, cs * SC:(cs + 1) * SC]
                # separate tiles per strip so the first matmul starts as soon
                # as strip 0 lands (one tile would add a whole-2MB dependency)
                hT_sb = []
                for k in range(16):
                    hk = a_sb_pool.tile([P, SC], f16, tag=f"hT{k}")
                    dma(out=hk, in_=hsrc[k * P:(k + 1) * P, :])
                    hT_sb.append(hk)
                kv_dst = t["a_kv_loc"] if use_collective else t["a_kv"][cs]
                ssq_ps = [a_ssq_pool.tile([1, SC], f32, tag=f"ssq{i}", name=f"ssq{i}")
                          for i in range(2)]

                def evict(psum, dst, dst_row, ssq_grp, ssq_start, ssq_stop,
                          rows=P, dt=f16):
                    val = a_ev_pool.tile([P, SC], dt, tag="val" + str(dt))
                    nc.vector.tensor_copy(val[0:rows, :], psum[0:rows, :])
                    dma(out=dst[dst_row:dst_row + rows, :], in_=val[0:rows, :])
                    if ssq_grp is not None:
                        sq = a_ev_pool.tile([P, SC], f16, tag="sq")
                        nc.scalar.activation(sq[:], psum[:], AF.Square)
                        MM(ssq_ps[ssq_grp][:], lhsT=ones_col_sb[:], rhs=sq[:],
                           start=ssq_start, stop=ssq_stop)

                # ---- kv strip (5 m-tiles: 4 c_kv + 1 rope, 640 cols) ----
                psums = [a_ps_pool.tile([P, SC], f32, tag=f"aps{i}", name=f"aps{i}")
                         for i in range(5)]
                for k in range(16):
                    wt = a_wkv_pool.tile([P, 640], f16, tag="awkv")
                    dma(out=wt, in_=t["kvawT_s"][k])
                    for ml in range(5):
                        MM(psums[ml][:], lhsT=wt[:, ml * P:(ml + 1) * P],
                           rhs=hT_sb[k][:], start=(k == 0), stop=(k == 15))
                for ml in range(4):
                    evict(psums[ml], kv_dst, ml * P, 1, ml == 0, ml == 3)
                evict(psums[4], kv_dst, KVL, None, False, False, rows=D_ROPE)
                # local 1/rms scale for the kv chunk (gathered as fp16 row)
                rs = a_row_pool.tile([1, SC], f32, tag="rs")
                nc.scalar.activation(rs[:], ssq_ps[1][:], AF.Sqrt,
                                     scale=1.0 / KVL, bias=eps_sb[:])
                rr = a_row_pool.tile([1, SC], f32, tag="rr")
                nc.vector.reciprocal_approx_fast(rr[:], rs[:])
                s_sb = a_row_pool.tile([1, SC], f16, tag="ssqe")
                nc.vector.tensor_copy(s_sb[:], rr[:])
                dma(out=kv_dst[KVL + D_ROPE:KVL + D_ROPE + 1, :], in_=s_sb[:])
                if use_collective and cs == 0:
                    nc.gpsimd.collective_compute(
                        "AllGather", mybir.AluOpType.bypass, replica_groups=RG,
                        ins=[t["a_kv_loc"].opt()], outs=[t["a_kv"].opt()])
                if cs == 0:
                    # kv up-proj weight preloads land in the stage-A DMA
                    # slack, before the gathers own the fabric
                    dma(out=kvbw_k_sb, in_=t["kvbw_k"].rearrange("(t p) n -> p t n", p=P))
                    dma(out=kvbw_v_sb, in_=t["kvbw_v"].rearrange("(t p) n -> p t n", p=P))

                # ---- q strips (3 x 4 m-tiles); chunk gathers: mg0, then mg1+mg2 ----
                for mg in range(3):
                    if use_collective:
                        q_dst, qrow0 = t["a_q_loc"], mg * 512
                    elif mg == 0:
                        q_dst, qrow0 = t["a_q0"][cs], 0
                    else:
                        q_dst, qrow0 = t["a_q1"][cs], (mg - 1) * 512
                    psums = [a_ps_pool.tile([P, SC], f32, tag=f"aps{i}", name=f"aps{i}")
                             for i in range(4)]
                    for k in range(16):
                        wt = a_wq_pool.tile([P, 512], f16, tag="awq")
                        dma(out=wt, in_=t["qawT_s"][k, mg])
                        for ml in range(4):
                            MM(psums[ml][:], lhsT=wt[:, ml * P:(ml + 1) * P],
                               rhs=hT_sb[k][:], start=(k == 0), stop=(k == 15))
                    for ml in range(4):
                        m = mg * 4 + ml
                        evict(psums[ml], q_dst, qrow0 + ml * P, 0, m == 0, m == 11,
                              dt=f8)
                    if use_collective and cs == 0 and mg == 0:
                        nc.gpsimd.collective_compute(
                            "AllGather", mybir.AluOpType.bypass, replica_groups=RG,
                            ins=[t["a_q_loc"][0:512, :].opt()],
                            outs=[t["a_q0"].opt()])
                rs = a_row_pool.tile([1, SC], f32, tag="rs")
                nc.scalar.activation(rs[:], ssq_ps[0][:], AF.Sqrt,
                                     scale=1.0 / QLR, bias=eps_sb[:])
                rr = a_row_pool.tile([1, SC], f32, tag="rr")
                nc.vector.reciprocal_approx_fast(rr[:], rs[:])
                # scale row shipped as fp8 (value, residual) pair rows
                s8 = a_row_pool.tile([1, SC], f8, tag="s8")
                nc.vector.tensor_copy(s8[:], rr[:])
                sdq = a_row_pool.tile([1, SC], f32, tag="sdq")
                nc.vector.tensor_copy(sdq[:], s8[:])
                sre = a_row_pool.tile([1, SC], f32, tag="sre")
                nc.vector.tensor_sub(sre[:], rr[:], sdq[:])
                s8r = a_row_pool.tile([1, SC], f8, tag="s8r")
                nc.vector.tensor_copy(s8r[:], sre[:])
                sc_dst = t["a_q_loc"][1536:1538, :] if use_collective \
                    else t["a_q1"][cs][1024:1026, :]
                dma(out=sc_dst[0:1, :], in_=s8[:])
                dma(out=sc_dst[1:2, :], in_=s8r[:])
                if use_collective and cs == 0:
                    nc.gpsimd.collective_compute(
                        "AllGather", mybir.AluOpType.bypass, replica_groups=RG,
                        ins=[t["a_q_loc"][512:1538, :].opt()],
                        outs=[t["a_q1"].opt()])

        # ---------------- persistent pools (post stage A) ----------------
        qk_stack = ExitStack()   # closed after stage C
        b_stack = ExitStack()    # closed after stage B
        pers_qk = qk_stack.enter_context(tc.tile_pool(name="pers_qk", bufs=1))
        pers_v = qk_stack.enter_context(tc.tile_pool(name="pers_v", bufs=1))
        q_npT = [pers_qk.tile([P, S], f16, tag=f"qnp{m}", name=f"qnp{m}") for m in range(NHL)]
        k_npT = [pers_qk.tile([P, S], f16, tag=f"knp{h}", name=f"knp{h}") for h in range(NHL)]
        # rope side in fp8 for DoubleRow score matmuls: k packs hold
        # (value, residual) plane pairs per 128-col block; _lo has the rope
        # dims on partitions 0-63 (even heads), _hi on 64-127 (odd heads),
        # zeros elsewhere so the off-half of the q operand is masked out.
        krp8_lo = pers_qk.tile([P, 16, 2, P], f8, tag="krp8l", name="krp8l")
        krp8_hi = pers_qk.tile([P, 16, 2, P], f8, tag="krp8h", name="krp8h")
        nc.vector.memset(krp8_lo[64:P, :, :, :], 0.0)
        nc.vector.memset(krp8_hi[0:64, :, :, :], 0.0)
        qrp8 = [pers_qk.tile([P, S], f8, tag=f"qrp8{m}", name=f"qrp8{m}")
                for m in range(2)]
        v_t = [pers_v.tile([P, NHL * D_V], f16, tag=f"v{st}", name=f"v{st}") for st in range(16)]

        bconsts = b_stack.enter_context(tc.tile_pool(name="bconsts", bufs=1))
        cosP_sb = bconsts.tile([P, S], f32)
        dma(out=cosP_sb, in_=t["cosP"])
        sinP_sb = bconsts.tile([P, S], f32)
        dma(out=sinP_sb, in_=t["sinP"])
        r128_sb = bconsts.tile([P, P], f16)
        dma(out=r128_sb, in_=t["r128t"])

        # ======== kv rms scales: broadcast-load the gathered fp16 scale row ====
        rpool = b_stack.enter_context(tc.tile_pool(name="rpool", bufs=1))
        kv_dt = t["a_kv"].tensor
        kv_roff = (KVL + D_ROPE) * SC
        bcast_rk = rpool.tile([P, S], f16, tag="brk")
        dma(out=bcast_rk.rearrange("p (c s) -> p c s", c=4),
            in_=bass.AP(tensor=kv_dt, offset=kv_roff,
                        ap=[[0, P], [KV_ROWS * SC, 4], [1, SC]]))
        # rkvT[p, st] = scale(seq st*128+p): row-to-column via tiny
        # K=1 transpose matmuls (an element-strided DMA here costs ~23us)
        rkv_row = rpool.tile([1, S], f16, tag="rkvrow")
        for c in range(4):
            dma(out=rkv_row[0:1, c * SC:(c + 1) * SC],
                in_=t["a_kv"][c, KVL + D_ROPE:KVL + D_ROPE + 1, :])
        rkvT = rpool.tile([P, 16], f32, tag="rkvT")
        with tc.tile_pool(name="rkv_ps", bufs=1, space="PSUM") as rkv_psp:
            rkv_ps = rkv_psp.tile([P, 16], f32)
            for st in range(16):
                MM(rkv_ps[:, st:st + 1], lhsT=rkv_row[0:1, st * P:(st + 1) * P],
                   rhs=ones_11[:], start=True, stop=True)
            nc.vector.tensor_copy(rkvT[:], rkv_ps[:])

        # ================ Stage B kv-pass (overlaps the q gathers) ============
        with tc.tile_pool(name="bk_s", bufs=4) as bk_s, \
             tc.tile_pool(name="bk_t", bufs=2) as bk_t, \
             tc.tile_pool(name="bk_ps", bufs=1, space="PSUM") as bk_ps, \
             tc.tile_pool(name="bk_ps2", bufs=2, space="PSUM") as bk_ps2:
            # hoist all chunk loads: they only need the kv gather, so they
            # land before the q gathers hog the fabric
            kva_sbs = []
            for c in range(4):
                kva_sb = bk_s.tile([P, 4, SC], f16, tag="kva")
                dma(out=kva_sb,
                    in_=t["a_kv"][c, 0:KVL, :].rearrange("(t p) s -> p t s", p=P))
                kva_sbs.append(kva_sb)
            # q_b weights queue behind the kva loads (needed later than them)
            dma(out=qbw_np_sb, in_=t["qbw_np"].rearrange("(t p) n -> p t n", p=P))
            dma(out=qbw_rp_sb, in_=t["qbw_rp"].rearrange("(t p) n -> p t n", p=P))
            for c in range(4):
                cols = slice(c * SC, (c + 1) * SC)
                kva_sb = kva_sbs[c]
                kps = [bk_ps.tile([P, SC], f32, tag=f"kps{h}", name=f"kps{h}")
                       for h in range(NHL)]
                for f in range(4):
                    for h in range(NHL):
                        MM(kps[h][:], lhsT=kvbw_k_sb[:, f, h * P:(h + 1) * P],
                           rhs=kva_sb[:, f, :], start=(f == 0), stop=(f == 3))
                for h in range(NHL):
                    nc.vector.tensor_mul(k_npT[h][:, cols], kps[h][:], bcast_rk[:, cols])
                for stl in range(4):
                    st = 4 * c + stl
                    vps = bk_ps2.tile([P, NHL * D_V], f32, tag="vps")
                    for f in range(4):
                        MM(vps[:], lhsT=kva_sb[:, f, stl * P:(stl + 1) * P],
                           rhs=kvbw_v_sb[:, f, :], start=(f == 0), stop=(f == 3))
                    nc.vector.tensor_scalar_mul(v_t[st][:], vps[:], rkvT[:, st:st + 1])
                # k_rope (loaded duplicated into both 64-partition halves)
                rp_f = bk_t.tile([P, SC], f16, tag="rp")
                kv_t_ = t["a_kv"].tensor
                rope_off = (c * KV_ROWS + KVL) * SC
                dma(out=rp_f[:],
                    in_=bass.AP(tensor=kv_t_, offset=rope_off,
                                ap=[[0, 2], [SC, D_ROPE], [1, SC]]))
                rot_ps = bk_ps2.tile([P, SC], f32, tag="rotk")
                MM(rot_ps[:], lhsT=r128_sb[:], rhs=rp_f[:], start=True, stop=True)
                t1 = bk_t.tile([P, SC], f32, tag="t1k")
                nc.vector.tensor_mul(t1[:], rot_ps[:], sinP_sb[:, cols])
                t2 = bk_t.tile([P, SC], f32, tag="t2k")
                nc.vector.tensor_mul(t2[:], rp_f[:], cosP_sb[:, cols])
                rope32 = bk_t.tile([P, SC], f32, tag="rope32")
                nc.vector.tensor_add(rope32[:], t1[:], t2[:])
                # fp8 packs: plane0 = value, plane1 = residual (k-side
                # compensation halves the rope quantization error)
                deq = bk_t.tile([P, SC], f32, tag="deq")
                res = bk_t.tile([P, SC], f32, tag="res")
                for pack, rlo in ((krp8_lo, slice(0, 64)), (krp8_hi, slice(64, P))):
                    dst0 = pack[rlo, 4 * c:4 * c + 4, 0, :]
                    src = rope32[rlo, :].rearrange("p (j m) -> p j m", j=4)
                    nc.vector.tensor_copy(dst0, src)
                    nc.vector.tensor_copy(deq[rlo, :].rearrange("p (j m) -> p j m", j=4), dst0)
                    nc.vector.tensor_sub(res[rlo, :], rope32[rlo, :], deq[rlo, :])
                    nc.vector.tensor_copy(pack[rlo, 4 * c:4 * c + 4, 1, :],
                                          res[rlo, :].rearrange("p (j m) -> p j m", j=4))

        # ======== q rms scales: rebuild fp32-ish scale from fp8 pair rows ======
        brq_v = rpool.tile([P, S], f8, tag="brqv")
        dma(out=brq_v.rearrange("p (c s) -> p c s", c=4),
            in_=bass.AP(tensor=t["a_q1"].tensor, offset=1024 * SC,
                        ap=[[0, P], [1026 * SC, 4], [1, SC]]))
        brq_r = rpool.tile([P, S], f8, tag="brqr")
        dma(out=brq_r.rearrange("p (c s) -> p c s", c=4),
            in_=bass.AP(tensor=t["a_q1"].tensor, offset=1025 * SC,
                        ap=[[0, P], [1026 * SC, 4], [1, SC]]))
        bcast_rq = rpool.tile([P, S], f32, tag="brq")
        nc.vector.tensor_add(bcast_rq[:], brq_v[:], brq_r[:])

        # ================ Stage B q-pass ================
        with tc.tile_pool(name="bq_s", bufs=3) as bq_s, \
             tc.tile_pool(name="bq_t", bufs=2) as bq_t, \
             tc.tile_pool(name="bq_ps", bufs=1, space="PSUM") as bq_ps, \
             tc.tile_pool(name="bq_ps2", bufs=2, space="PSUM") as bq_ps2:
            for c in range(4):
                cols = slice(c * SC, (c + 1) * SC)
                qps = [bq_ps.tile([P, SC], f32, tag=f"qps{m}", name=f"qps{m}")
                       for m in range(6)]
                for f in range(12):
                    qa_f8 = bq_s.tile([P, SC], f8, tag="qa8")
                    if f < 4:
                        qsrc = t["a_q0"][c, f * P:(f + 1) * P, :]
                    else:
                        qsrc = t["a_q1"][c, (f - 4) * P:(f - 3) * P, :]
                    dma(out=qa_f8, in_=qsrc)
                    qa_f = bq_s.tile([P, SC], f16, tag="qa")
                    nc.vector.tensor_copy(qa_f[:], qa_f8[:])
                    for m in range(4):
                        MM(qps[m][:], lhsT=qbw_np_sb[:, f, m * P:(m + 1) * P],
                           rhs=qa_f[:], start=(f == 0), stop=(f == 11))
                    for m2 in range(2):
                        MM(qps[4 + m2][:], lhsT=qbw_rp_sb[:, f, m2 * P:(m2 + 1) * P],
                           rhs=qa_f[:], start=(f == 0), stop=(f == 11))
                for m in range(4):
                    nc.vector.tensor_mul(q_npT[m][:, cols], qps[m][:], bcast_rq[:, cols])
                for m2 in range(2):
                    x_sb = bq_t.tile([P, SC], f16, tag="x")
                    nc.vector.tensor_mul(x_sb[:], qps[4 + m2][:], bcast_rq[:, cols])
                    rot_ps = bq_ps2.tile([P, SC], f32, tag="rot")
                    MM(rot_ps[:], lhsT=r128_sb[:], rhs=x_sb[:], start=True, stop=True)
                    t1 = bq_t.tile([P, SC], f32, tag="t1")
                    nc.vector.tensor_mul(t1[:], rot_ps[:], sinP_sb[:, cols])
                    t2 = bq_t.tile([P, SC], f32, tag="t2")
                    nc.vector.tensor_mul(t2[:], x_sb[:], cosP_sb[:, cols])
                    nc.vector.tensor_add(qrp8[m2][:, cols], t1[:], t2[:])

        b_stack.close()

        # o-proj weights stream in during stage C, well before stage D
        dma(out=ow_sb, in_=t["owT"].rearrange("(t p) n -> p t n", p=P))

        # ================ Stage C: attention ================
        pers_at = ec(tc.tile_pool(name="pers_at", bufs=1, side="right"))
        at_onT = [pers_at.tile([P, S], f16, tag=f"aon{h}", name=f"aon{h}") for h in range(NHL)]
        with tc.tile_pool(name="c_pt", bufs=4) as c_pt, \
             tc.tile_pool(name="c_r", bufs=3) as c_r, \
             tc.tile_pool(name="c_sc", bufs=2, space="PSUM") as c_sc, \
             tc.tile_pool(name="c_at", bufs=3, space="PSUM") as c_at, \
             tc.tile_pool(name="c_dn", bufs=1, space="PSUM") as c_dn, \
             tc.tile_pool(name="c_bc", bufs=2, space="PSUM") as c_bc:
            # all in-flight denominators share one PSUM bank: rotating
            # [1,512] slots at base partitions 0/32/64
            dn_all = c_dn.tile([65, 512], f32, tag="dn", name="dn_all")

            def emit_norm(h, qcols, at_ps, dn_ps):
                dn_sb = c_r.tile([1, 512], f32, tag="dnsb")
                nc.scalar.copy(dn_sb[:], dn_ps)
                rec32 = c_r.tile([1, 512], f32, tag="rec32")
                nc.vector.reciprocal_approx_fast(rec32[:], dn_sb[:])
                rec = c_r.tile([1, 512], f16, tag="rec")
                nc.vector.tensor_copy(rec[:], rec32[:])
                bc_ps = c_bc.tile([P, 512], f32, tag="bc")
                MM(bc_ps[:], lhsT=ones_row16[:], rhs=rec[:], start=True, stop=True)
                bc_sb = c_r.tile([P, 512], f32, tag="bcs")
                nc.scalar.copy(bc_sb[:], bc_ps[:])
                nc.vector.tensor_mul(at_onT[h][:, qcols], at_ps[:], bc_sb[:])

            pending = None
            for h in range(NHL):
                krp8 = krp8_lo if h % 2 == 0 else krp8_hi
                qrp8_t = qrp8[h // 2]
                for Q in range(4):
                    qcols = slice(Q * 512, (Q + 1) * 512)
                    at_ps = c_at.tile([P, 512], f32, tag="at")
                    slot = ((h * 4 + Q) % 3) * 32
                    dn_ps = dn_all[slot:slot + 1, :]
                    jmax = 4 * Q + 3
                    for j in range(jmax + 1):
                        jp = j - 4 * Q
                        lo = max(jp, 0) * P
                        qsl = slice(Q * 512 + lo, (Q + 1) * 512)
                        ksl = slice(j * P, (j + 1) * P)
                        sc_ps = c_sc.tile([P, 512], f32, tag="sc")
                        MM(sc_ps[:, lo:], lhsT=k_npT[h][:, ksl], rhs=q_npT[h][:, qsl],
                           start=True, stop=False)
                        # rope part: fp8 DoubleRow, (value, residual) planes on
                        # the k side, q plane streamed twice via 0-stride AP
                        q_ap = qrp8_t[:, qsl]
                        q_dr = bass.AP(tensor=q_ap.tensor, offset=q_ap.offset,
                                       ap=[q_ap.ap[0], [0, 2], [1, 512 - lo]])
                        MM(sc_ps[:, lo:], lhsT=krp8[:, j, :, :], rhs=q_dr,
                           start=False, stop=True, perf_mode=DR,
                           skip_group_check=True)
                        pt = c_pt.tile([P, 512], f16, tag="pt")
                        nc.scalar.activation(pt[:, lo:], sc_ps[:, lo:], AF.Exp,
                                             scale=SCALING, bias=shift_sb[:])
                        if jp >= 0:
                            nc.vector.tensor_mul(pt[:, lo:lo + P], pt[:, lo:lo + P],
                                                 triu_sb[:])
                        MM(at_ps[:, lo:], lhsT=v_t[j][:, h * D_V:(h + 1) * D_V],
                           rhs=pt[:, lo:], start=(j == 0), stop=(j == jmax))
                        MM(dn_ps[:, lo:], lhsT=ones_col_sb[:], rhs=pt[:, lo:],
                           start=(j == 0), stop=(j == jmax))
                    # normalization for the PREVIOUS (h,Q): its bc matmul sits
                    # behind this (h,Q)'s scores in the in-order PE queue, so
                    # the 3.4us vector reciprocal never stalls the PE
                    if pending is not None:
                        emit_norm(*pending)
                    pending = (h, qcols, at_ps, dn_ps)
            emit_norm(*pending)

        qk_stack.close()

        # ================ Stage D: o-proj partial ================
        with tc.tile_pool(name="d_o", bufs=2) as d_o, \
             tc.tile_pool(name="d_ps", bufs=2, space="PSUM") as d_ps:
            for qt in range(16):
                out_sb = d_o.tile([P, H], f16, tag="out")
                for hc in range(4):
                    psum = d_ps.tile([P, 512], f32, tag="dps")
                    for f in range(4):
                        MM(psum[:], lhsT=at_onT[f][:, qt * P:(qt + 1) * P],
                           rhs=ow_sb[:, f, hc * 512:(hc + 1) * 512],
                           start=(f == 0), stop=(f == 3))
                    nc.scalar.copy(out_sb[:, hc * 512:(hc + 1) * 512], psum[:])
                dma(out=t["o_part"][qt * P:(qt + 1) * P, :], in_=out_sb[:])


# ---------------- host side ----------------
_CACHED = {}


def _get_program(use_collective=True):
    key = use_collective
    if key not in _CACHED:
        _CACHED[key] = _build_program(use_collective)
    return _CACHED[key]


def _host_consts():
    inv_freq = 1.0 / (ROPE_THETA ** (np.arange(0, D_ROPE, 2, dtype=np.float32) / D_ROPE))
    ti = np.arange(S, dtype=np.float32)
    ang = np.outer(ti, inv_freq)
    emb = np.concatenate([ang, ang], axis=-1)          # [S, 64]
    cosT = np.cos(emb).T.astype(np.float32)            # [64, S]
    sinT = np.sin(emb).T.astype(np.float32)
    cosP = np.vstack([cosT, cosT])                     # [128, S]
    sinP = np.vstack([sinT, sinT])
    r64 = np.zeros((D_ROPE, D_ROPE), np.float16)
    hlf = D_ROPE // 2
    for i in range(hlf):
        r64[i, i + hlf] = -1.0
        r64[i + hlf, i] = 1.0
    r128 = np.zeros((P, P), np.float16)
    r128[:D_ROPE, :D_ROPE] = r64
    r128[D_ROPE:, D_ROPE:] = r64
    r128t = np.ascontiguousarray(r128.T)
    kk, qq = np.meshgrid(np.arange(P), np.arange(P), indexing="ij")
    triu = (kk <= qq).astype(np.float16)
    return cosP, sinP, r128t, triu


def make_in_maps(hidden_states, q_a_w, q_a_ln_w, q_b_w, kv_a_w, kv_a_ln_w,
                 kv_b_w, o_w, use_collective=True):
    f, f16_ = np.float32, np.float16
    hidden_states = np.asarray(hidden_states, f)
    q_b_eff = (np.asarray(q_b_w, f) * np.asarray(q_a_ln_w, f)[None, :]).astype(f16_)
    kv_b_eff = (np.asarray(kv_b_w, f) * np.asarray(kv_a_ln_w, f)[None, :]).astype(f16_)
    qawT = np.asarray(q_a_w, f).T.astype(f16_)         # [H, QLR]
    kvawT_pad = np.zeros((H, 5 * P), f16_)
    kvawT_pad[:, :KVL + D_ROPE] = np.asarray(kv_a_w, f).T.astype(f16_)
    qawT_s = np.ascontiguousarray(
        qawT.reshape(16, P, 3, 512).transpose(0, 2, 1, 3))
    kvawT_s = np.ascontiguousarray(kvawT_pad.reshape(16, P, 640))
    cosP, sinP, r128t, triu = _host_consts()
    ones_col = np.ones((P, 1), f16_)
    ones_row = np.ones((1, P), f)

    in_maps = []
    for core in range(N_CORES):
        b, g = divmod(core, 4)
        heads = range(NHL * g, NHL * (g + 1))
        if use_collective:
            hT = np.ascontiguousarray(hidden_states[b, g * SC:(g + 1) * SC, :].T.astype(f16_))
        else:
            hT = np.ascontiguousarray(hidden_states[b].T.astype(f16_))
        qbw_np = np.ascontiguousarray(np.concatenate(
            [q_b_eff[D_QK * hh:D_QK * hh + D_NOPE] for hh in heads], 0).T)
        qbw_rp = np.ascontiguousarray(np.concatenate(
            [q_b_eff[D_QK * hh + D_NOPE:D_QK * (hh + 1)] for hh in heads], 0).T)
        kvbw_k = np.ascontiguousarray(np.concatenate(
            [kv_b_eff[(D_NOPE + D_V) * hh:(D_NOPE + D_V) * hh + D_NOPE]
             for hh in heads], 0).T)
        kvbw_v = np.ascontiguousarray(np.concatenate(
            [kv_b_eff[(D_NOPE + D_V) * hh + D_NOPE:(D_NOPE + D_V) * (hh + 1)]
             for hh in heads], 0).T)
        owT = np.ascontiguousarray(
            np.asarray(o_w, f)[:, g * NHL * D_V:(g + 1) * NHL * D_V].T.astype(f16_))
        in_maps.append(dict(
            hT=hT, qawT_s=qawT_s, kvawT_s=kvawT_s, qbw_np=qbw_np,
            qbw_rp=qbw_rp, kvbw_k=kvbw_k, kvbw_v=kvbw_v, owT=owT,
            cosP=cosP, sinP=sinP, r128t=r128t, triu=triu,
            ones_col=ones_col, ones_row=ones_row))
    return in_maps


def kernel(**inputs):
    use_collective = True
    nc = _get_program(use_collective)
    in_maps = make_in_maps(**inputs, use_collective=use_collective)
    res = run_bass_kernel_spmd(nc, in_maps, core_ids=list(range(N_CORES)))
    out = np.zeros((B, S, H), np.float32)
    for core in range(N_CORES):
        b = core // 4
        out[b] += res.results[core]["o_part"].astype(np.float32)
    return out


# revision 52
# speedup vs baseline: 1.1588x; 1.0092x over previous
"""DeepseekV3 MLA attention on 8 Trainium2 NeuronCores (Bass/Tile).

Sharding: core = (batch b, head-group g); b = core//4, g = core%4.
Each core handles 4 of 16 heads for one batch.
 - q_a / kv_a low-rank projections are sequence-sharded within each 4-core
   batch group (each core computes a 512-column chunk of q_a^T / kv_a^T),
   then AllGathered.  The kv gather fires right after the kv strip; the q
   gather is split into three row-chunk gathers, each fired as soon as its
   512-row strip is evicted, so gathers pipeline with stage-A compute.
 - Stage B runs the kv up-projection first (it only needs the early kv
   gather), hiding the q gathers behind it; all stage-B/D weights are
   prefetched into SBUF before the collectives start so the PE never
   stalls on DMA during a gather.
 - q_b / kv_b up-projections + causal attention computed per-core for its
   4 heads, everything in "transposed" (feature-on-partition) layout.
 - o-proj partial output per core ([S, H] fp16, contraction over the 512
   local v-dims); host sums the 4 partials per batch in fp32.

All matmul operands are fp16 (2-byte streaming runs the PE at full rate;
fp32 streams at half rate), accumulation is always fp32 in PSUM.  Softmax
uses exp(scale*s - 2) with no row-max subtraction (global shift cancels in
the ratio; score range is bounded well inside fp16), with the denominator
produced by a ones-vector matmul over the transposed probability tiles.
"""
import sys

sys.path.insert(0, "/opt/trn_rl_repo")

import numpy as np

import concourse.bass as bass
import concourse.tile as tile
from concourse import bacc, mybir
from concourse.bass_utils import run_bass_kernel_spmd

# ---- problem constants (hardcoded per contract) ----
B, S, H = 2, 2048, 2048
NH, D_NOPE, D_ROPE, D_V = 16, 128, 64, 128
D_QK = D_NOPE + D_ROPE
QLR, KVL = 1536, 512
SCALING = float(D_QK) ** -0.5
EXP_SHIFT = -2.0   # global exponent shift; cancels in softmax ratio
EPS = 1e-6
ROPE_THETA = 10000.0

NHL = 4            # heads per core
N_CORES = 8
SC = S // 4        # seq chunk per core for the a-projections (512)
P = 128
KV_ROWS = KVL + D_ROPE + 1   # 577: c_kv rows, k_rope rows, kv sumsq row
Q_ROWS = QLR + 1             # 1537: q_a rows, q sumsq row

f32 = mybir.dt.float32
f32r = mybir.dt.float32r
f16 = mybir.dt.float16
f8 = mybir.dt.float8e4
DR = mybir.MatmulPerfMode.DoubleRow
AF = mybir.ActivationFunctionType
RG = [[0, 1, 2, 3], [4, 5, 6, 7]]


def _build_program(use_collective=True):
    nc = bacc.Bacc("TRN2", target_bir_lowering=False, debug=False,
                   num_devices=N_CORES)

    def din(name, shape, dt=f16):
        return nc.dram_tensor(name, list(shape), dt, kind="ExternalInput").ap()

    # per-core inputs (host-prepped layouts, fp16)
    hT = din("hT", [H, S if not use_collective else SC])
    qawT_s = din("qawT_s", [16, 3, P, 512])      # q_a_w.T strips [k, mg, 128, 512]
    kvawT_s = din("kvawT_s", [16, P, 640])       # kv_a_w.T strips (cols padded 576->640)
    qbw_np = din("qbw_np", [QLR, NHL * D_NOPE])  # [1536, 512]
    qbw_rp = din("qbw_rp", [QLR, NHL * D_ROPE])  # [1536, 256]
    kvbw_k = din("kvbw_k", [KVL, NHL * D_NOPE])  # [512, 512]
    kvbw_v = din("kvbw_v", [KVL, NHL * D_V])     # [512, 512]
    owT = din("owT", [NHL * D_V, H])             # [512, 2048]
    cosP = din("cosP", [P, S], f32)              # vstack(cos.T, cos.T)
    sinP = din("sinP", [P, S], f32)
    r128t = din("r128t", [P, P])                 # rotate-half matrix (transposed, block-diag)
    triu = din("triu", [P, P])                   # causal mask for diagonal blocks
    ones_col = din("ones_col", [P, 1])           # lhsT for denominator matmul
    ones_row = din("ones_row", [1, P], f32)      # lhsT for partition-broadcast matmul (f32r)

    o_part = nc.dram_tensor("o_part", [S, H], f16, kind="ExternalOutput").ap()

    # scratch (fp16): kv block gathered first, q chunks pipelined after
    a_kv_loc = nc.dram_tensor("a_kv_loc", [KV_ROWS, SC], f16).ap()
    a_q_loc = nc.dram_tensor("a_q_loc", [1538, SC], f8).ap()
    a_kvA = nc.dram_tensor("a_kvA", [4, 384, SC], f16).ap()
    a_kvB = nc.dram_tensor("a_kvB", [4, 193, SC], f16).ap()
    a_q0 = nc.dram_tensor("a_q0", [4, 512, SC], f8).ap()
    a_q1 = nc.dram_tensor("a_q1", [4, 1026, SC], f8).ap()

    with tile.TileContext(nc) as tc:
        _emit(tc, nc, use_collective, dict(
            hT=hT, qawT_s=qawT_s, kvawT_s=kvawT_s, qbw_np=qbw_np,
            qbw_rp=qbw_rp, kvbw_k=kvbw_k, kvbw_v=kvbw_v, owT=owT,
            cosP=cosP, sinP=sinP, r128t=r128t, triu=triu,
            ones_col=ones_col, ones_row=ones_row,
            o_part=o_part, a_kv_loc=a_kv_loc, a_q_loc=a_q_loc,
            a_kvA=a_kvA, a_kvB=a_kvB, a_q0=a_q0, a_q1=a_q1))
    nc.compile()
    return nc


def _emit(tc, nc, use_collective, t):
    from contextlib import ExitStack

    dma = nc.sync.dma_start
    MM = nc.tensor.matmul

    with ExitStack() as stack:
        ec = stack.enter_context

        consts = ec(tc.tile_pool(name="consts", bufs=1))
        ones_col_sb = consts.tile([P, 1], f16)
        dma(out=ones_col_sb, in_=t["ones_col"])
        ones_row_sb = consts.tile([1, P], f32r)
        dma(out=ones_row_sb, in_=t["ones_row"].bitcast(f32r))
        triu_sb = consts.tile([P, P], f16)
        dma(out=triu_sb, in_=t["triu"])
        shift_sb = consts.tile([P, 1], f32)
        nc.vector.memset(shift_sb[:], EXP_SHIFT)
        eps_sb = consts.tile([1, 1], f32)
        nc.vector.memset(eps_sb[:], EPS)
        ones_11 = consts.tile([1, 1], f16)
        nc.vector.memset(ones_11[:], 1.0)
        ones_row16 = consts.tile([1, P], f16)
        nc.vector.memset(ones_row16[:], 1.0)

        # stage-B/D weight tiles (DMAs fired from inside stage A)
        pre_w = ec(tc.tile_pool(name="pre_w", bufs=1))
        kvbw_k_sb = pre_w.tile([P, 4, NHL * D_NOPE], f16)
        kvbw_v_sb = pre_w.tile([P, 4, NHL * D_V], f16)
        qbw_np_sb = pre_w.tile([P, 12, NHL * D_NOPE], f16)
        qbw_rp_sb = pre_w.tile([P, 12, NHL * D_ROPE], f16)
        ow_sb = pre_w.tile([P, 4, H], f16)

        # ================ Stage A: a-projections (kv strip first) ================
        n_chunk = 1 if use_collective else 4
        with tc.tile_pool(name="a_sb", bufs=1) as a_sb_pool, \
             tc.tile_pool(name="a_wkv", bufs=16) as a_wkv_pool, \
             tc.tile_pool(name="a_wq", bufs=24) as a_wq_pool, \
             tc.tile_pool(name="a_ev", bufs=6) as a_ev_pool, \
             tc.tile_pool(name="a_row", bufs=2) as a_row_pool, \
             tc.tile_pool(name="a_ps", bufs=1, space="PSUM") as a_ps_pool, \
             tc.tile_pool(name="a_ssq", bufs=1, space="PSUM") as a_ssq_pool:
            for cs in range(n_chunk):
                if use_collective:
                    hsrc = t["hT"]
                else:
                    hsrc = t["hT"][:, cs * SC:(cs + 1) * SC]
                # separate tiles per strip so the first matmul starts as soon
                # as strip 0 lands (one tile would add a whole-2MB dependency)
                hT_sb = []
                for k in range(16):
                    hk = a_sb_pool.tile([P, SC], f16, tag=f"hT{k}")
                    dma(out=hk, in_=hsrc[k * P:(k + 1) * P, :])
                    hT_sb.append(hk)
                ssq_ps = [a_ssq_pool.tile([1, SC], f32, tag=f"ssq{i}", name=f"ssq{i}")
                          for i in range(2)]

                def evict(psum, dst, dst_row, ssq_grp, ssq_start, ssq_stop,
                          rows=P, dt=f16):
                    val = a_ev_pool.tile([P, SC], dt, tag="val" + str(dt))
                    nc.vector.tensor_copy(val[0:rows, :], psum[0:rows, :])
                    dma(out=dst[dst_row:dst_row + rows, :], in_=val[0:rows, :])
                    if ssq_grp is not None:
                        sq = a_ev_pool.tile([P, SC], f16, tag="sq")
                        nc.scalar.activation(sq[:], psum[:], AF.Square)
                        MM(ssq_ps[ssq_grp][:], lhsT=ones_col_sb[:], rhs=sq[:],
                           start=ssq_start, stop=ssq_stop)

                # ---- kv strip in two m-groups so the gather fires early ----
                kvw_sb = []
                for k in range(16):
                    wk = a_sb_pool.tile([P, 640], f16, tag=f"kvw{k}")
                    dma(out=wk, in_=t["kvawT_s"][k])
                    kvw_sb.append(wk)
                kv_dstA = t["a_kv_loc"] if use_collective else t["a_kvA"][cs]
                kv_dstB = t["a_kv_loc"][384:577, :] if use_collective \
                    else t["a_kvB"][cs]
                psums = [a_ps_pool.tile([P, SC], f32, tag=f"aps{i}", name=f"aps{i}")
                         for i in range(3)]
                for k in range(16):
                    for ml in range(3):
                        MM(psums[ml][:], lhsT=kvw_sb[k][:, ml * P:(ml + 1) * P],
                           rhs=hT_sb[k][:], start=(k == 0), stop=(k == 15))
                for ml in range(3):
                    evict(psums[ml], kv_dstA, ml * P, 1, ml == 0, False)
                if use_collective and cs == 0:
                    nc.gpsimd.collective_compute(
                        "AllGather", mybir.AluOpType.bypass, replica_groups=RG,
                        ins=[t["a_kv_loc"][0:384, :].opt()],
                        outs=[t["a_kvA"].opt()])
                psums = [a_ps_pool.tile([P, SC], f32, tag=f"aps{3 + i}", name=f"aps{3 + i}")
                         for i in range(2)]
                for k in range(16):
                    for ml in range(2):
                        MM(psums[ml][:], lhsT=kvw_sb[k][:, (3 + ml) * P:(4 + ml) * P],
                           rhs=hT_sb[k][:], start=(k == 0), stop=(k == 15))
                evict(psums[0], kv_dstB, 0, 1, False, True)
                evict(psums[1], kv_dstB, P, None, False, False, rows=D_ROPE)
                # local 1/rms scale for the kv chunk (gathered as fp16 row)
                rs = a_row_pool.tile([1, SC], f32, tag="rs")
                nc.scalar.activation(rs[:], ssq_ps[1][:], AF.Sqrt,
                                     scale=1.0 / KVL, bias=eps_sb[:])
                rr = a_row_pool.tile([1, SC], f32, tag="rr")
                nc.vector.reciprocal_approx_fast(rr[:], rs[:])
                s_sb = a_row_pool.tile([1, SC], f16, tag="ssqe")
                nc.vector.tensor_copy(s_sb[:], rr[:])
                dma(out=kv_dstB[192:193, :], in_=s_sb[:])
                if use_collective and cs == 0:
                    nc.gpsimd.collective_compute(
                        "AllGather", mybir.AluOpType.bypass, replica_groups=RG,
                        ins=[t["a_kv_loc"][384:577, :].opt()],
                        outs=[t["a_kvB"].opt()])
                if cs == 0:
                    # kv up-proj weight preloads land in the stage-A DMA
                    # slack, before the gathers own the fabric
                    dma(out=kvbw_k_sb, in_=t["kvbw_k"].rearrange("(t p) n -> p t n", p=P))
                    dma(out=kvbw_v_sb, in_=t["kvbw_v"].rearrange("(t p) n -> p t n", p=P))

                # ---- q strips (3 x 4 m-tiles); chunk gathers: mg0, then mg1+mg2 ----
                for mg in range(3):
                    if use_collective:
                        q_dst, qrow0 = t["a_q_loc"], mg * 512
                    elif mg == 0:
                        q_dst, qrow0 = t["a_q0"][cs], 0
                    else:
                        q_dst, qrow0 = t["a_q1"][cs], (mg - 1) * 512
                    psums = [a_ps_pool.tile([P, SC], f32, tag=f"aps{i}", name=f"aps{i}")
                             for i in range(4)]
                    for k in range(16):
                        wt = a_wq_pool.tile([P, 512], f16, tag="awq")
                        dma(out=wt, in_=t["qawT_s"][k, mg])
                        for ml in range(4):
                            MM(psums[ml][:], lhsT=wt[:, ml * P:(ml + 1) * P],
                               rhs=hT_sb[k][:], start=(k == 0), stop=(k == 15))
                    for ml in range(4):
                        m = mg * 4 + ml
                        evict(psums[ml], q_dst, qrow0 + ml * P, 0, m == 0, m == 11,
                              dt=f8)
                    if use_collective and cs == 0 and mg == 0:
                        nc.gpsimd.collective_compute(
                            "AllGather", mybir.AluOpType.bypass, replica_groups=RG,
                            ins=[t["a_q_loc"][0:512, :].opt()],
                            outs=[t["a_q0"].opt()])
                rs = a_row_pool.tile([1, SC], f32, tag="rs")
                nc.scalar.activation(rs[:], ssq_ps[0][:], AF.Sqrt,
                                     scale=1.0 / QLR, bias=eps_sb[:])
                rr = a_row_pool.tile([1, SC], f32, tag="rr")
                nc.vector.reciprocal_approx_fast(rr[:], rs[:])
                # scale row shipped as fp8 (value, residual) pair rows
                s8 = a_row_pool.tile([1, SC], f8, tag="s8")
                nc.vector.tensor_copy(s8[:], rr[:])
                sdq = a_row_pool.tile([1, SC], f32, tag="sdq")
                nc.vector.tensor_copy(sdq[:], s8[:])
                sre = a_row_pool.tile([1, SC], f32, tag="sre")
                nc.vector.tensor_sub(sre[:], rr[:], sdq[:])
                s8r = a_row_pool.tile([1, SC], f8, tag="s8r")
                nc.vector.tensor_copy(s8r[:], sre[:])
                sc_dst = t["a_q_loc"][1536:1538, :] if use_collective \
                    else t["a_q1"][cs][1024:1026, :]
                dma(out=sc_dst[0:1, :], in_=s8[:])
                dma(out=sc_dst[1:2, :], in_=s8r[:])
                if use_collective and cs == 0:
                    nc.gpsimd.collective_compute(
                        "AllGather", mybir.AluOpType.bypass, replica_groups=RG,
                        ins=[t["a_q_loc"][512:1538, :].opt()],
                        outs=[t["a_q1"].opt()])

        # ---------------- persistent pools (post stage A) ----------------
        qk_stack = ExitStack()   # closed after stage C
        b_stack = ExitStack()    # closed after stage B
        pers_qk = qk_stack.enter_context(tc.tile_pool(name="pers_qk", bufs=1))
        pers_v = qk_stack.enter_context(tc.tile_pool(name="pers_v", bufs=1))
        q_npT = [pers_qk.tile([P, S], f16, tag=f"qnp{m}", name=f"qnp{m}") for m in range(NHL)]
        k_npT = [pers_qk.tile([P, S], f16, tag=f"knp{h}", name=f"knp{h}") for h in range(NHL)]
        # rope side in fp8 for DoubleRow score matmuls: k packs hold
        # (value, residual) plane pairs per 128-col block; _lo has the rope
        # dims on partitions 0-63 (even heads), _hi on 64-127 (odd heads),
        # zeros elsewhere so the off-half of the q operand is masked out.
        krp8_lo = pers_qk.tile([P, 16, 2, P], f8, tag="krp8l", name="krp8l")
        krp8_hi = pers_qk.tile([P, 16, 2, P], f8, tag="krp8h", name="krp8h")
        nc.vector.memset(krp8_lo[64:P, :, :, :], 0.0)
        nc.vector.memset(krp8_hi[0:64, :, :, :], 0.0)
        qrp8 = [pers_qk.tile([P, S], f8, tag=f"qrp8{m}", name=f"qrp8{m}")
                for m in range(2)]
        v_t = [pers_v.tile([P, NHL * D_V], f16, tag=f"v{st}", name=f"v{st}") for st in range(16)]

        bconsts = b_stack.enter_context(tc.tile_pool(name="bconsts", bufs=1))
        cosP_sb = bconsts.tile([P, S], f32)
        dma(out=cosP_sb, in_=t["cosP"])
        sinP_sb = bconsts.tile([P, S], f32)
        dma(out=sinP_sb, in_=t["sinP"])
        r128_sb = bconsts.tile([P, P], f16)
        dma(out=r128_sb, in_=t["r128t"])

        # ======== kv rms scales: broadcast-load the gathered fp16 scale row ====
        rpool = b_stack.enter_context(tc.tile_pool(name="rpool", bufs=1))
        kv_dt = t["a_kvB"].tensor
        kv_roff = 192 * SC
        bcast_rk = rpool.tile([P, S], f16, tag="brk")
        dma(out=bcast_rk.rearrange("p (c s) -> p c s", c=4),
            in_=bass.AP(tensor=kv_dt, offset=kv_roff,
                        ap=[[0, P], [193 * SC, 4], [1, SC]]))
        # rkvT[p, st] = scale(seq st*128+p): row-to-column via tiny
        # K=1 transpose matmuls (an element-strided DMA here costs ~23us)
        rkv_row = rpool.tile([1, S], f16, tag="rkvrow")
        for c in range(4):
            dma(out=rkv_row[0:1, c * SC:(c + 1) * SC],
                in_=t["a_kvB"][c, 192:193, :])
        rkvT = rpool.tile([P, 16], f32, tag="rkvT")
        with tc.tile_pool(name="rkv_ps", bufs=1, space="PSUM") as rkv_psp:
            rkv_ps = rkv_psp.tile([P, 16], f32)
            for st in range(16):
                MM(rkv_ps[:, st:st + 1], lhsT=rkv_row[0:1, st * P:(st + 1) * P],
                   rhs=ones_11[:], start=True, stop=True)
            nc.vector.tensor_copy(rkvT[:], rkv_ps[:])

        # ================ Stage B kv-pass (overlaps the q gathers) ============
        with tc.tile_pool(name="bk_s", bufs=4) as bk_s, \
             tc.tile_pool(name="bk_t", bufs=2) as bk_t, \
             tc.tile_pool(name="bk_ps", bufs=1, space="PSUM") as bk_ps, \
             tc.tile_pool(name="bk_ps2", bufs=2, space="PSUM") as bk_ps2:
            # hoist all chunk loads: they only need the kv gather, so they
            # land before the q gathers hog the fabric
            kva_sbs = []
            for c in range(4):
                kva_sb = bk_s.tile([P, 4, SC], f16, tag="kva")
                dma(out=kva_sb[:, 0:3, :],
                    in_=t["a_kvA"][c].rearrange("(t p) s -> p t s", p=P))
                dma(out=kva_sb[:, 3, :], in_=t["a_kvB"][c, 0:P, :])
                kva_sbs.append(kva_sb)
            # q_b weights queue behind the kva loads (needed later than them)
            dma(out=qbw_np_sb, in_=t["qbw_np"].rearrange("(t p) n -> p t n", p=P))
            dma(out=qbw_rp_sb, in_=t["qbw_rp"].rearrange("(t p) n -> p t n", p=P))
            for c in range(4):
                cols = slice(c * SC, (c + 1) * SC)
                kva_sb = kva_sbs[c]
                kps = [bk_ps.tile([P, SC], f32, tag=f"kps{h}", name=f"kps{h}")
                       for h in range(NHL)]
                for f in range(4):
                    for h in range(NHL):
                        MM(kps[h][:], lhsT=kvbw_k_sb[:, f, h * P:(h + 1) * P],
                           rhs=kva_sb[:, f, :], start=(f == 0), stop=(f == 3))
                for h in range(NHL):
                    nc.vector.tensor_mul(k_npT[h][:, cols], kps[h][:], bcast_rk[:, cols])
                for stl in range(4):
                    st = 4 * c + stl
                    vps = bk_ps2.tile([P, NHL * D_V], f32, tag="vps")
                    for f in range(4):
                        MM(vps[:], lhsT=kva_sb[:, f, stl * P:(stl + 1) * P],
                           rhs=kvbw_v_sb[:, f, :], start=(f == 0), stop=(f == 3))
                    nc.vector.tensor_scalar_mul(v_t[st][:], vps[:], rkvT[:, st:st + 1])
                # k_rope (loaded duplicated into both 64-partition halves)
                rp_f = bk_t.tile([P, SC], f16, tag="rp")
                kv_t_ = t["a_kvB"].tensor
                rope_off = (c * 193 + P) * SC
                dma(out=rp_f[:],
                    in_=bass.AP(tensor=kv_t_, offset=rope_off,
                                ap=[[0, 2], [SC, D_ROPE], [1, SC]]))
                rot_ps = bk_ps2.tile([P, SC], f32, tag="rotk")
                MM(rot_ps[:], lhsT=r128_sb[:], rhs=rp_f[:], start=True, stop=True)
                t1 = bk_t.tile([P, SC], f32, tag="t1k")
                nc.vector.tensor_mul(t1[:], rot_ps[:], sinP_sb[:, cols])
                t2 = bk_t.tile([P, SC], f32, tag="t2k")
                nc.vector.tensor_mul(t2[:], rp_f[:], cosP_sb[:, cols])
                rope32 = bk_t.tile([P, SC], f32, tag="rope32")
                nc.vector.tensor_add(rope32[:], t1[:], t2[:])
                # fp8 packs: plane0 = value, plane1 = residual (k-side
                # compensation halves the rope quantization error)
                deq = bk_t.tile([P, SC], f32, tag="deq")
                res = bk_t.tile([P, SC], f32, tag="res")
                for pack, rlo in ((krp8_lo, slice(0, 64)), (krp8_hi, slice(64, P))):
                    dst0 = pack[rlo, 4 * c:4 * c + 4, 0, :]
                    src = rope32[rlo, :].rearrange("p (j m) -> p j m", j=4)
                    nc.vector.tensor_copy(dst0, src)
                    nc.vector.tensor_copy(deq[rlo, :].rearrange("p (j m) -> p j m", j=4), dst0)
                    nc.vector.tensor_sub(res[rlo, :], rope32[rlo, :], deq[rlo, :])
                    nc.vector.tensor_copy(pack[rlo, 4 * c:4 * c + 4, 1, :],
                                          res[rlo, :].rearrange("p (j m) -> p j m", j=4))

        # ======== q rms scales: rebuild fp32-ish scale from fp8 pair rows ======
        brq_v = rpool.tile([P, S], f8, tag="brqv")
        dma(out=brq_v.rearrange("p (c s) -> p c s", c=4),
            in_=bass.AP(tensor=t["a_q1"].tensor, offset=1024 * SC,
                        ap=[[0, P], [1026 * SC, 4], [1, SC]]))
        brq_r = rpool.tile([P, S], f8, tag="brqr")
        dma(out=brq_r.rearrange("p (c s) -> p c s", c=4),
            in_=bass.AP(tensor=t["a_q1"].tensor, offset=1025 * SC,
                        ap=[[0, P], [1026 * SC, 4], [1, SC]]))
        bcast_rq = rpool.tile([P, S], f32, tag="brq")
        nc.vector.tensor_add(bcast_rq[:], brq_v[:], brq_r[:])

        # ================ Stage B q-pass ================
        with tc.tile_pool(name="bq_s", bufs=4) as bq_s, \
             tc.tile_pool(name="bq_t", bufs=2) as bq_t, \
             tc.tile_pool(name="bq_ps", bufs=1, space="PSUM") as bq_ps, \
             tc.tile_pool(name="bq_ps2", bufs=2, space="PSUM") as bq_ps2:
            for c in range(4):
                cols = slice(c * SC, (c + 1) * SC)
                qps = [bq_ps.tile([P, SC], f32, tag=f"qps{m}", name=f"qps{m}")
                       for m in range(6)]
                for f in range(12):
                    qa_f8 = bq_s.tile([P, SC], f8, tag="qa8")
                    if f < 4:
                        qsrc = t["a_q0"][c, f * P:(f + 1) * P, :]
                    else:
                        qsrc = t["a_q1"][c, (f - 4) * P:(f - 3) * P, :]
                    dma(out=qa_f8, in_=qsrc)
                    qa_f = bq_s.tile([P, SC], f16, tag="qa")
                    nc.vector.tensor_copy(qa_f[:], qa_f8[:])
                    for m in range(4):
                        MM(qps[m][:], lhsT=qbw_np_sb[:, f, m * P:(m + 1) * P],
                           rhs=qa_f[:], start=(f == 0), stop=(f == 11))
                    for m2 in range(2):
                        MM(qps[4 + m2][:], lhsT=qbw_rp_sb[:, f, m2 * P:(m2 + 1) * P],
                           rhs=qa_f[:], start=(f == 0), stop=(f == 11))
                for m in range(4):
                    nc.vector.tensor_mul(q_npT[m][:, cols], qps[m][:], bcast_rq[:, cols])
                for m2 in range(2):
                    x_sb = bq_t.tile([P, SC], f16, tag="x")
                    nc.vector.tensor_mul(x_sb[:], qps[4 + m2][:], bcast_rq[:, cols])
                    rot_ps = bq_ps2.tile([P, SC], f32, tag="rot")
                    MM(rot_ps[:], lhsT=r128_sb[:], rhs=x_sb[:], start=True, stop=True)
                    t1 = bq_t.tile([P, SC], f32, tag="t1")
                    nc.vector.tensor_mul(t1[:], rot_ps[:], sinP_sb[:, cols])
                    t2 = bq_t.tile([P, SC], f32, tag="t2")
                    nc.vector.tensor_mul(t2[:], x_sb[:], cosP_sb[:, cols])
                    nc.vector.tensor_add(qrp8[m2][:, cols], t1[:], t2[:])

        b_stack.close()

        # o-proj weights stream in during stage C, well before stage D
        dma(out=ow_sb, in_=t["owT"].rearrange("(t p) n -> p t n", p=P))

        # ================ Stage C: attention ================
        pers_at = ec(tc.tile_pool(name="pers_at", bufs=1, side="right"))
        at_onT = [pers_at.tile([P, S], f16, tag=f"aon{h}", name=f"aon{h}") for h in range(NHL)]
        with tc.tile_pool(name="c_pt", bufs=4) as c_pt, \
             tc.tile_pool(name="c_r", bufs=3) as c_r, \
             tc.tile_pool(name="c_sc", bufs=2, space="PSUM") as c_sc, \
             tc.tile_pool(name="c_at", bufs=3, space="PSUM") as c_at, \
             tc.tile_pool(name="c_dn", bufs=1, space="PSUM") as c_dn, \
             tc.tile_pool(name="c_bc", bufs=2, space="PSUM") as c_bc:
            # all in-flight denominators share one PSUM bank: rotating
            # [1,512] slots at base partitions 0/32/64
            dn_all = c_dn.tile([65, 512], f32, tag="dn", name="dn_all")

            def emit_norm(h, qcols, at_ps, dn_ps):
                dn_sb = c_r.tile([1, 512], f32, tag="dnsb")
                nc.scalar.copy(dn_sb[:], dn_ps)
                rec32 = c_r.tile([1, 512], f32, tag="rec32")
                nc.vector.reciprocal_approx_fast(rec32[:], dn_sb[:])
                rec = c_r.tile([1, 512], f16, tag="rec")
                nc.vector.tensor_copy(rec[:], rec32[:])
                bc_ps = c_bc.tile([P, 512], f32, tag="bc")
                MM(bc_ps[:], lhsT=ones_row16[:], rhs=rec[:], start=True, stop=True)
                bc_sb = c_r.tile([P, 512], f32, tag="bcs")
                nc.scalar.copy(bc_sb[:], bc_ps[:])
                nc.vector.tensor_mul(at_onT[h][:, qcols], at_ps[:], bc_sb[:])

            pending = None
            for h in range(NHL):
                krp8 = krp8_lo if h % 2 == 0 else krp8_hi
                qrp8_t = qrp8[h // 2]
                for Q in range(4):
                    qcols = slice(Q * 512, (Q + 1) * 512)
                    at_ps = c_at.tile([P, 512], f32, tag="at")
                    slot = ((h * 4 + Q) % 3) * 32
                    dn_ps = dn_all[slot:slot + 1, :]
                    jmax = 4 * Q + 3
                    for j in range(jmax + 1):
                        jp = j - 4 * Q
                        lo = max(jp, 0) * P
                        qsl = slice(Q * 512 + lo, (Q + 1) * 512)
                        ksl = slice(j * P, (j + 1) * P)
                        sc_ps = c_sc.tile([P, 512], f32, tag="sc")
                        MM(sc_ps[:, lo:], lhsT=k_npT[h][:, ksl], rhs=q_npT[h][:, qsl],
                           start=True, stop=False)
                        # rope part: fp8 DoubleRow, (value, residual) planes on
                        # the k side, q plane streamed twice via 0-stride AP
                        q_ap = qrp8_t[:, qsl]
                        q_dr = bass.AP(tensor=q_ap.tensor, offset=q_ap.offset,
                                       ap=[q_ap.ap[0], [0, 2], [1, 512 - lo]])
                        MM(sc_ps[:, lo:], lhsT=krp8[:, j, :, :], rhs=q_dr,
                           start=False, stop=True, perf_mode=DR,
                           skip_group_check=True)
                        pt = c_pt.tile([P, 512], f16, tag="pt")
                        nc.scalar.activation(pt[:, lo:], sc_ps[:, lo:], AF.Exp,
                                             scale=SCALING, bias=shift_sb[:])
                        if jp >= 0:
                            nc.vector.tensor_mul(pt[:, lo:lo + P], pt[:, lo:lo + P],
                                                 triu_sb[:])
                        MM(at_ps[:, lo:], lhsT=v_t[j][:, h * D_V:(h + 1) * D_V],
                           rhs=pt[:, lo:], start=(j == 0), stop=(j == jmax))
                        MM(dn_ps[:, lo:], lhsT=ones_col_sb[:], rhs=pt[:, lo:],
                           start=(j == 0), stop=(j == jmax))
                    # normalization for the PREVIOUS (h,Q): its bc matmul sits
                    # behind this (h,Q)'s scores in the in-order PE queue, so
                    # the 3.4us vector reciprocal never stalls the PE
                    if pending is not None:
                        emit_norm(*pending)
                    pending = (h, qcols, at_ps, dn_ps)
            emit_norm(*pending)

        qk_stack.close()

        # ================ Stage D: o-proj partial ================
        with tc.tile_pool(name="d_o", bufs=2) as d_o, \
             tc.tile_pool(name="d_ps", bufs=2, space="PSUM") as d_ps:
            for qt in range(16):
                out_sb = d_o.tile([P, H], f16, tag="out")
                for hc in range(4):
                    psum = d_ps.tile([P, 512], f32, tag="dps")
                    for f in range(4):
                        MM(psum[:], lhsT=at_onT[f][:, qt * P:(qt + 1) * P],
                           rhs=ow_sb[:, f, hc * 512:(hc + 1) * 512],
                           start=(f == 0), stop=(f == 3))
                    nc.scalar.copy(out_sb[:, hc * 512:(hc + 1) * 512], psum[:])
                dma(out=t["o_part"][qt * P:(qt + 1) * P, :], in_=out_sb[:])


# ---------------- host side ----------------
_CACHED = {}


def _get_program(use_collective=True):
    key = use_collective
    if key not in _CACHED:
        _CACHED[key] = _build_program(use_collective)
    return _CACHED[key]


def _host_consts():
    inv_freq = 1.0 / (ROPE_THETA ** (np.arange(0, D_ROPE, 2, dtype=np.float32) / D_ROPE))
    ti = np.arange(S, dtype=np.float32)
    ang = np.outer(ti, inv_freq)
    emb = np.concatenate([ang, ang], axis=-1)          # [S, 64]
    cosT = np.cos(emb).T.astype(np.float32)            # [64, S]
    sinT = np.sin(emb).T.astype(np.float32)
    cosP = np.vstack([cosT, cosT])                     # [128, S]
    sinP = np.vstack([sinT, sinT])
    r64 = np.zeros((D_ROPE, D_ROPE), np.float16)
    hlf = D_ROPE // 2
    for i in range(hlf):
        r64[i, i + hlf] = -1.0
        r64[i + hlf, i] = 1.0
    r128 = np.zeros((P, P), np.float16)
    r128[:D_ROPE, :D_ROPE] = r64
    r128[D_ROPE:, D_ROPE:] = r64
    r128t = np.ascontiguousarray(r128.T)
    kk, qq = np.meshgrid(np.arange(P), np.arange(P), indexing="ij")
    triu = (kk <= qq).astype(np.float16)
    return cosP, sinP, r128t, triu


def make_in_maps(hidden_states, q_a_w, q_a_ln_w, q_b_w, kv_a_w, kv_a_ln_w,
                 kv_b_w, o_w, use_collective=True):
    f, f16_ = np.float32, np.float16
    hidden_states = np.asarray(hidden_states, f)
    q_b_eff = (np.asarray(q_b_w, f) * np.asarray(q_a_ln_w, f)[None, :]).astype(f16_)
    kv_b_eff = (np.asarray(kv_b_w, f) * np.asarray(kv_a_ln_w, f)[None, :]).astype(f16_)
    qawT = np.asarray(q_a_w, f).T.astype(f16_)         # [H, QLR]
    kvawT_pad = np.zeros((H, 5 * P), f16_)
    kvawT_pad[:, :KVL + D_ROPE] = np.asarray(kv_a_w, f).T.astype(f16_)
    qawT_s = np.ascontiguousarray(
        qawT.reshape(16, P, 3, 512).transpose(0, 2, 1, 3))
    kvawT_s = np.ascontiguousarray(kvawT_pad.reshape(16, P, 640))
    cosP, sinP, r128t, triu = _host_consts()
    ones_col = np.ones((P, 1), f16_)
    ones_row = np.ones((1, P), f)

    in_maps = []
    for core in range(N_CORES):
        b, g = divmod(core, 4)
        heads = range(NHL * g, NHL * (g + 1))
        if use_collective:
            hT = np.ascontiguousarray(hidden_states[b, g * SC:(g + 1) * SC, :].T.astype(f16_))
        else:
            hT = np.ascontiguousarray(hidden_states[b].T.astype(f16_))
        qbw_np = np.ascontiguousarray(np.concatenate(
            [q_b_eff[D_QK * hh:D_QK * hh + D_NOPE] for hh in heads], 0).T)
        qbw_rp = np.ascontiguousarray(np.concatenate(
            [q_b_eff[D_QK * hh + D_NOPE:D_QK * (hh + 1)] for hh in heads], 0).T)
        kvbw_k = np.ascontiguousarray(np.concatenate(
            [kv_b_eff[(D_NOPE + D_V) * hh:(D_NOPE + D_V) * hh + D_NOPE]
             for hh in heads], 0).T)
        kvbw_v = np.ascontiguousarray(np.concatenate(
            [kv_b_eff[(D_NOPE + D_V) * hh + D_NOPE:(D_NOPE + D_V) * (hh + 1)]
             for hh in heads], 0).T)
        owT = np.ascontiguousarray(
            np.asarray(o_w, f)[:, g * NHL * D_V:(g + 1) * NHL * D_V].T.astype(f16_))
        in_maps.append(dict(
            hT=hT, qawT_s=qawT_s, kvawT_s=kvawT_s, qbw_np=qbw_np,
            qbw_rp=qbw_rp, kvbw_k=kvbw_k, kvbw_v=kvbw_v, owT=owT,
            cosP=cosP, sinP=sinP, r128t=r128t, triu=triu,
            ones_col=ones_col, ones_row=ones_row))
    return in_maps


def kernel(**inputs):
    use_collective = True
    nc = _get_program(use_collective)
    in_maps = make_in_maps(**inputs, use_collective=use_collective)
    res = run_bass_kernel_spmd(nc, in_maps, core_ids=list(range(N_CORES)))
    out = np.zeros((B, S, H), np.float32)
    for core in range(N_CORES):
        b = core // 4
        out[b] += res.results[core]["o_part"].astype(np.float32)
    return out
